# revision 1
# baseline (speedup 1.0000x reference)
"""GAT (2-layer graph attention) Trainium2 Bass kernel, 8-core SPMD.

Sharding: data-parallel over batch (2) x row-blocks (4) -> 8 cores.
Core c handles batch b=c//4, output rows R=[512*(c%4), 512*(c%4+1)).

Key algebra: with z = s_src[i]+s_tgt[j], the GAT edge weight
exp(leaky_relu(z, 0.2)) = max(exp(z), exp(0.2 z)).  Softmax rows are
invariant to a per-row scale, so dividing row i by exp(s_src[i]) gives
unnormalized weights F[j,i] = adj[i,j] * D[j] * max(W[j], g[i]) with
  W[j] = exp(0.8 s_tgt[j]),  D[j] = exp(0.2 s_tgt[j]),  g[i] = exp(-0.8 s_src[i])
-- no per-element transcendentals.  D[j] is folded into the stationary
matmul operand (proj rows, plus a D-valued ones-column so the softmax
denominator falls out as matmul row 64).  The n x n inner work is one
fused DVE op per 128x512 tile: (g_bcast max W[j]) * adjT[j,i], feeding
the TensorE numerator matmul in bf16.

Layer-1 outputs are exchanged within each batch group of 4 cores via a
single AllGather of (proj2^T | s_tgt2) so layer 2 stays row-local.
"""

import os
import sys

for _p in ("/opt/trn_rl_repo", "/root/.axon_site/_ro/trn_rl_repo"):
    if os.path.isdir(_p) and _p not in sys.path:
        sys.path.insert(0, _p)

import numpy as np

import concourse.bass as bass
import concourse.bacc as bacc
import concourse.mybir as mybir
from concourse import tile
from concourse.bass_utils import run_bass_kernel_spmd

F32 = mybir.dt.float32
BF16 = mybir.dt.bfloat16
AF = mybir.ActivationFunctionType
ALU = mybir.AluOpType

BS, N, FIN = 2, 2048, 128
H1, F1 = 8, 64
RB = 512          # row block per core
NJT = N // 128    # 16 j-tiles
NIT = RB // 128   # 4 i-tiles in the row block
NCORES = 8
GROUPS = [[0, 1, 2, 3], [4, 5, 6, 7]]


def build_nc():
    nc = bacc.Bacc("TRN2", target_bir_lowering=False, debug=False,
                   num_devices=NCORES)

    # ---- per-core DRAM I/O ----
    d_x = nc.declare_dram_parameter("xb", [N, FIN], F32, isOutput=False)
    d_xr = nc.declare_dram_parameter("xbr", [RB, FIN], F32, isOutput=False)
    d_adj = nc.declare_dram_parameter("adjr", [RB, N], F32, isOutput=False)
    d_w1 = nc.declare_dram_parameter("w1", [H1 * F1, FIN], F32, isOutput=False)
    d_ws1 = nc.declare_dram_parameter("wskip1", [H1 * F1, FIN], F32, isOutput=False)
    d_as1 = nc.declare_dram_parameter("asrc1", [H1, F1], F32, isOutput=False)
    d_at1 = nc.declare_dram_parameter("atgt1", [H1, F1], F32, isOutput=False)
    d_b1 = nc.declare_dram_parameter("b1", [H1 * F1], F32, isOutput=False)
    d_w2 = nc.declare_dram_parameter("w2", [F1, H1 * F1], F32, isOutput=False)
    d_ws2 = nc.declare_dram_parameter("wskip2", [F1, H1 * F1], F32, isOutput=False)
    d_as2 = nc.declare_dram_parameter("asrc2", [1, F1], F32, isOutput=False)
    d_at2 = nc.declare_dram_parameter("atgt2", [1, F1], F32, isOutput=False)
    d_b2 = nc.declare_dram_parameter("b2", [F1], F32, isOutput=False)
    # output: transposed row-block out^T [64, 512] (host transposes back)
    d_out = nc.declare_dram_parameter("outT", [F1, RB], F32, isOutput=True)

    with tile.TileContext(nc) as tc:
        with (
            tc.tile_pool(name="persist", bufs=1) as P,
            tc.tile_pool(name="work", bufs=4) as WK,
            tc.tile_pool(name="gbp", bufs=3) as GB,
            tc.tile_pool(name="ps", bufs=3, space="PSUM") as PS,
            tc.tile_pool(name="psnum", bufs=3, space="PSUM") as PSN,
            tc.tile_pool(name="pst", bufs=2, space="PSUM") as PST,
            tc.tile_pool(name="dram", bufs=1, space="DRAM") as DR,
        ):
            # ============ loads (transposes via strided DMA) ================
            # spread across the two HWDGE queues (SP=sync, ACT=scalar)
            xT = P.tile([128, N], F32, tag="xT")
            nc.sync.dma_start(xT[:], d_x.rearrange("j c -> c j"))
            xTb = P.tile([128, N], BF16, tag="xTb")
            nc.vector.tensor_copy(xTb[:], xT[:])
            xrTf = P.tile([128, RB], F32, tag="xrTf")
            nc.scalar.dma_start(xrTf[:], d_xr.rearrange("j c -> c j"))
            xrTb = P.tile([128, RB], BF16, tag="xrTb")
            nc.vector.tensor_copy(xrTb[:], xrTf[:])

            ws1Tf = P.tile([128, H1 * F1], F32, tag="ws1Tf")
            nc.scalar.dma_start(ws1Tf[:], d_ws1.rearrange("f c -> c f"))
            ws1Tb = P.tile([128, H1 * F1], BF16, tag="ws1Tb")
            nc.vector.tensor_copy(ws1Tb[:], ws1Tf[:])

            w1n = P.tile([128, 4, FIN], F32, tag="w1n")
            nc.sync.dma_start(w1n[:], d_w1.rearrange("(k p) c -> p k c", p=128))
            w2n = P.tile([F1, H1 * F1], F32, tag="w2n")
            nc.scalar.dma_start(w2n[:], d_w2[:, :])
            ws2n = P.tile([F1, H1 * F1], F32, tag="ws2n")
            nc.scalar.dma_start(ws2n[:], d_ws2[:, :])

            b1f = P.tile([128, 4], F32, tag="b1f")
            nc.sync.dma_start(b1f[:], d_b1.rearrange("(k p) -> p k", p=128))
            b2f = P.tile([F1, 1], F32, tag="b2f")
            nc.sync.dma_start(b2f[:], d_b2.ap().rearrange("(f o) -> f o", o=1))
            b2sb = P.tile([F1, 1], BF16, tag="b2sb")
            nc.vector.tensor_copy(b2sb[:], b2f[:])

            # a-vector tables, transposed on load: [128, 8] (col h = head h,
            # duplicated in both partition halves so matmul base matches W1)
            a1sT = P.tile([128, H1], F32, tag="a1sT")
            nc.sync.dma_start(a1sT[0:F1, :], d_as1.rearrange("h f -> f h"))
            nc.sync.dma_start(a1sT[F1:2 * F1, :], d_as1.rearrange("h f -> f h"))
            a1tT = P.tile([128, H1], F32, tag="a1tT")
            nc.sync.dma_start(a1tT[0:F1, :], d_at1.rearrange("h f -> f h"))
            nc.sync.dma_start(a1tT[F1:2 * F1, :], d_at1.rearrange("h f -> f h"))
            a2p = P.tile([F1, 2], F32, tag="a2p")
            nc.scalar.dma_start(a2p[:, 0:1], d_as2.rearrange("o f -> f o"))
            nc.scalar.dma_start(a2p[:, 1:2], d_at2.rearrange("o f -> f o"))

            ones1b = P.tile([1, 128], BF16, tag="ones1b")
            nc.vector.memset(ones1b[:], 1.0)

            # identity (bf16) for PE transposes
            onesq = P.tile([128, 128], BF16, tag="onesq")
            nc.vector.memset(onesq[:], 1.0)
            ident = P.tile([128, 128], BF16, tag="ident")
            nc.gpsimd.affine_select(ident[:], onesq[:], [[-1, 128]],
                                    ALU.is_equal, 0.0, base=0,
                                    channel_multiplier=1)
            identf = P.tile([128, 128], F32, tag="identf")
            onesqf = P.tile([128, 128], F32, tag="onesqf")
            nc.vector.memset(onesqf[:], 1.0)
            nc.gpsimd.affine_select(identf[:], onesqf[:], [[-1, 128]],
                                    ALU.is_equal, 0.0, base=0,
                                    channel_multiplier=1)
            # W1^T / W2^T / Wskip2^T (bf16) via PE transposes of natural tiles
            w1Tb = P.tile([128, H1 * F1], BF16, tag="w1Tb")
            for kt in range(4):
                ps_w = PS.tile([128, 512], F32, tag="ps")
                nc.tensor.transpose(ps_w[0:128, 0:128], w1n[:, kt, :], identf[:])
                nc.vector.tensor_copy(w1Tb[:, kt * 128:(kt + 1) * 128],
                                      ps_w[0:128, 0:128])
            w2Tb = P.tile([128, 4, F1], BF16, tag="w2Tb")
            ws2Tb = P.tile([128, 4, F1], BF16, tag="ws2Tb")
            for kt in range(4):
                ps_w = PS.tile([128, 512], F32, tag="ps")
                nc.tensor.transpose(ps_w[0:128, 0:64],
                                    w2n[:, kt * 128:(kt + 1) * 128],
                                    identf[0:64, 0:64])
                nc.vector.tensor_copy(w2Tb[:, kt, :], ps_w[0:128, 0:64])
            for kt in range(4):
                ps_w = PS.tile([128, 512], F32, tag="ps")
                nc.tensor.transpose(ps_w[0:128, 0:64],
                                    ws2n[:, kt * 128:(kt + 1) * 128],
                                    identf[0:64, 0:64])
                nc.vector.tensor_copy(ws2Tb[:, kt, :], ps_w[0:128, 0:64])

            # selector tiles sel_h [16, 128] (row h ones) for g broadcasts
            ones16 = P.tile([16, 128], BF16, tag="ones16")
            nc.vector.memset(ones16[:], 1.0)
            sel = P.tile([16, H1 * 128], BF16, tag="sel")
            for h in range(H1):
                nc.gpsimd.affine_select(sel[:, h * 128:(h + 1) * 128],
                                        ones16[:], [[0, 128]], ALU.is_equal,
                                        0.0, base=-h, channel_multiplier=1)

            # ============ adjacency: cast-DMA + PE transpose ================
            adjRb = P.tile([128, 4, N], BF16, tag="adjRb")
            for it in range(2):
                nc.gpsimd.dma_start(adjRb[:, it, :],
                                    d_adj[it * 128:(it + 1) * 128, :])
            for it in range(2, 4):
                adjRf = P.tile([128, N], F32, tag="adjRf")
                nc.sync.dma_start(adjRf[:],
                                  d_adj[it * 128:(it + 1) * 128, :])
                nc.vector.tensor_copy(adjRb[:, it, :], adjRf[:])
            adjT = [P.tile([128, RB], BF16, tag=f"adjT{j}", name=f"adjTs{j}")
                    for j in range(NJT)]
            for jt in range(NJT):
                ps_t = PST.tile([128, 512], BF16, tag="pst")
                for it in range(NIT):
                    nc.tensor.transpose(ps_t[:, it * 128:(it + 1) * 128],
                                        adjRb[:, it, jt * 128:(jt + 1) * 128],
                                        ident[:])
                if jt % 2 == 0:
                    nc.vector.tensor_copy(adjT[jt][:], ps_t[:])
                else:
                    nc.scalar.activation(adjT[jt][:], ps_t[:], AF.Copy)

            # ============ small exact fp32 matmuls ==========================
            # w1tilde [c=128, 16]: col h = W1_h^T a_src1[h], col 8+h tgt
            ps_wt = PS.tile([128, 512], F32, tag="ps")
            for h in range(H1):
                kt, pr = (h * F1) // 128, (h * F1) % 128
                w1slc = w1n[pr:pr + F1, kt, :]
                nc.tensor.matmul(ps_wt[0:128, h:h + 1], w1slc,
                                 a1sT[pr:pr + F1, h:h + 1])
                nc.tensor.matmul(ps_wt[0:128, 8 + h:9 + h], w1slc,
                                 a1tT[pr:pr + F1, h:h + 1])
            w1t = P.tile([128, 16], F32, tag="w1t")
            nc.vector.tensor_copy(w1t[:], ps_wt[0:128, 0:16])

            # S1T [j(128 x 16 chunks), 16] = x @ w1tilde
            ps_s1t = PS.tile([128, 512], F32, tag="ps")
            for jc in range(NJT):
                nc.tensor.matmul(ps_s1t[0:128, jc * 16:(jc + 1) * 16],
                                 xT[:, jc * 128:(jc + 1) * 128], w1t[:])
            s1T = P.tile([128, NJT * 16], F32, tag="s1T")
            nc.vector.tensor_copy(s1T[:], ps_s1t[0:128, 0:NJT * 16])
            Wvf = P.tile([128, NJT * 16], F32, tag="Wvf")
            nc.scalar.activation(Wvf[:], s1T[:], AF.Exp, scale=0.8)
            Dvf = P.tile([128, NJT * 16], F32, tag="Dvf")
            nc.scalar.activation(Dvf[:], s1T[:], AF.Exp, scale=0.2)


            # s_src rows for our block -> g (bf16) [16, 512]
            ps_s1r = PS.tile([128, 512], F32, tag="ps")
            nc.tensor.matmul(ps_s1r[0:16, 0:RB], w1t[:], xrTf[:])
            g1b = P.tile([16, RB], BF16, tag="g1b")
            nc.scalar.activation(g1b[:], ps_s1r[0:16, 0:RB], AF.Exp, scale=-0.8)

            # ============ layer-1 skip:  (x_R @ Wskip1^T)^T  ================
            skipTb = P.tile([128, 4, RB], BF16, tag="skipTb")
            for pr in range(4):
                ps_sk = PS.tile([128, 512], F32, tag="ps")
                nc.tensor.matmul(ps_sk[0:128, 0:RB],
                                 ws1Tb[:, pr * 128:(pr + 1) * 128], xrTb[:])
                nc.scalar.activation(skipTb[:, pr, :], ps_sk[0:128, 0:RB],
                                     AF.Identity, bias=b1f[:, pr:pr + 1])

            # ============ proj1 (+ ones col, + D fold on gpsimd) ============
            p1e = P.tile([128, NJT, 8 * 66], BF16, tag="p1e")
            nc.vector.memset(
                p1e[:].rearrange("p j (h q) -> p j h q", q=66)[:, :, :, 64:65],
                1.0)
            for jt in range(NJT):
                ps_p = PS.tile([128, 512], F32, tag="ps")
                nc.tensor.matmul(ps_p[0:128, 0:512],
                                 xTb[:, jt * 128:(jt + 1) * 128], w1Tb[:])
                dst = p1e[:, jt, :].rearrange("p (h q) -> p h q", q=66)
                src = ps_p[0:128, 0:512].rearrange("p (h q) -> p h q", q=64)
                if jt % 2 == 0:
                    nc.vector.tensor_copy(dst[:, :, 0:64], src)
                else:
                    nc.scalar.activation(dst[:, :, 0:64], src, AF.Copy)

            # ============ layer-1 head loop =================================
            numb = P.tile([128, 4, RB], BF16, tag="numb")
            recbs = []
            for h in range(H1):
                ps_g = PS.tile([128, 512], F32, tag="ps")
                nc.tensor.matmul(ps_g[0:128, 0:RB],
                                 sel[:, h * 128:(h + 1) * 128], g1b[:])
                gbh = GB.tile([128, RB], BF16, tag="gb")
                nc.scalar.activation(gbh[:], ps_g[0:128, 0:RB], AF.Copy)

                numT = PSN.tile([65, 512], F32, tag="numT")
                for jt in range(NJT):
                    col = slice(jt * 16 + 8 + h, jt * 16 + 9 + h)
                    q = WK.tile([128, RB], BF16, tag="q")
                    Ft = WK.tile([128, RB], BF16, tag="F")
                    eng = nc.gpsimd if (jt % 3 == 2 or jt == 7) else nc.vector
                    eng.tensor_scalar(q[:], gbh[:], Wvf[:, col], Dvf[:, col],
                                      ALU.max, ALU.mult)
                    eng.tensor_tensor(Ft[:], q[:], adjT[jt][:], ALU.mult)
                    nc.tensor.matmul(numT[0:65, 0:RB],
                                     p1e[:, jt, h * 66:h * 66 + 65], Ft[:],
                                     start=(jt == 0), stop=(jt == NJT - 1))
                den_h = P.tile([1, RB], F32, tag=f"den{h}")
                nc.scalar.activation(den_h[:], numT[64:65, 0:RB], AF.Copy)
                rec_h = P.tile([1, RB], F32, tag=f"rec{h}")
                nc.vector.reciprocal_approx_fast(rec_h[:], den_h[:])
                recb_h = P.tile([1, RB], BF16, tag=f"recb{h}")
                nc.vector.tensor_copy(recb_h[:], rec_h[:])
                recbs.append(recb_h)
                nc.scalar.activation(numb[(h % 2) * 64:(h % 2) * 64 + 64, h // 2, :],
                                     numT[0:64, 0:RB], AF.Copy)

            # h_out^T = elu(num/den + (skip + b1)), kept bf16, per pair so
            # late pairs overlap earlier heads' compute
            houtb = P.tile([128, 4, RB], BF16, tag="houtb")
            for pr in range(4):
                rdb = GB.tile([128, RB], BF16, tag="gb")
                ps_r = PS.tile([128, 512], F32, tag="ps")
                nc.tensor.matmul(ps_r[0:64, 0:RB], ones1b[0:1, 0:64],
                                 recbs[2 * pr][:])
                nc.tensor.matmul(ps_r[64:128, 0:RB], ones1b[0:1, 0:64],
                                 recbs[2 * pr + 1][:])
                nc.scalar.activation(rdb[:], ps_r[0:128, 0:RB], AF.Copy)
                hpre = WK.tile([128, RB], BF16, tag="hpre")
                nc.vector.tensor_mul(hpre[:], numb[:, pr, :], rdb[:])
                u = WK.tile([128, RB], BF16, tag="u")
                nc.vector.tensor_add(u[:], hpre[:], skipTb[:, pr, :])
                m0 = WK.tile([128, RB], BF16, tag="hpre")
                nc.vector.tensor_scalar(m0[:], u[:], 0.0, None, ALU.min)
                e = WK.tile([128, RB], BF16, tag="e")
                nc.scalar.activation(e[:], m0[:], AF.Exp)
                nc.vector.scalar_tensor_tensor(
                    houtb[:, pr, :], e[:], -1.0, u[:], ALU.add, ALU.max)

            # ============ layer-2 local pieces ==============================
            ps_w2 = PS.tile([128, 512], F32, tag="ps")
            for kt in range(4):
                nc.tensor.matmul(ps_w2[0:128, kt * 2:kt * 2 + 2],
                                 w2n[:, kt * 128:(kt + 1) * 128], a2p[:],
                                 start=True, stop=True)
            w2tb = P.tile([128, 8], BF16, tag="w2tb")
            nc.vector.tensor_copy(w2tb[:], ps_w2[0:128, 0:8])

            # S2: s_src2 -> psum row 0, s_tgt2 -> psum row 32
            ps_s2 = PS.tile([128, 512], F32, tag="ps")
            for kt in range(4):
                nc.tensor.matmul(ps_s2[0:1, 0:RB], w2tb[:, kt * 2:kt * 2 + 1],
                                 houtb[:, kt, :], start=(kt == 0), stop=(kt == 3))
            for kt in range(4):
                nc.tensor.matmul(ps_s2[32:33, 0:RB], w2tb[:, kt * 2 + 1:kt * 2 + 2],
                                 houtb[:, kt, :], start=(kt == 0), stop=(kt == 3))
            g2row = P.tile([1, RB], BF16, tag="g2row")
            nc.scalar.activation(g2row[:], ps_s2[0:1, 0:RB], AF.Exp, scale=-0.8)
            stg2 = P.tile([1, RB], F32, tag="stg2")
            nc.scalar.activation(stg2[:], ps_s2[32:33, 0:RB], AF.Copy)

            # proj2^T local [64, 512] in bf16 for the gather
            ps_p2 = PS.tile([128, 512], F32, tag="ps")
            for kt in range(4):
                nc.tensor.matmul(ps_p2[0:64, 0:RB], w2Tb[:, kt, :],
                                 houtb[:, kt, :], start=(kt == 0), stop=(kt == 3))
            p2Tb = P.tile([F1, RB], BF16, tag="p2Tb")
            nc.scalar.activation(p2Tb[:], ps_p2[0:64, 0:RB], AF.Copy)

            # ============ AllGather within batch group ======================
            # gin2 bf16 [4, 65, 128]: rows 0..63 = proj2^T slices, row 64 =
            # s_tgt2 (bf16 value -- 4e-3 total error verified in mock)
            gin = DR.tile([4, F1 + 1, 128], BF16)
            nc.sync.dma_start(
                gin[:, 0:F1, :].rearrange("s f p -> f s p"),
                p2Tb[:].rearrange("f (s p) -> f s p", p=128))
            stg2b = P.tile([1, RB], BF16, tag="stg2b")
            nc.vector.tensor_copy(stg2b[:], stg2[:])
            nc.sync.dma_start(
                gin[:, F1:F1 + 1, :].rearrange("s o p -> o s p"),
                stg2b[:].rearrange("o (s p) -> o s p", p=128))
            gout = DR.tile([4, 4, F1 + 1, 128], BF16)
            nc.gpsimd.collective_compute(
                "AllGather", ALU.bypass, replica_groups=GROUPS,
                ins=[gin.opt()], outs=[gout.opt()])

            # ============ layer-2 attention =================================
            p2e = P.tile([128, NJT, F1 + 1], BF16, tag="p2e")
            nc.sync.dma_start(
                p2e[:],
                gout.rearrange("c s f p -> p (c s) f"))
            # the denominator column must be 1
            nc.vector.memset(p2e[:, :, F1:F1 + 1], 1.0)
            st2Tb = P.tile([128, 4, 4], BF16, tag="st2Tb")
            nc.scalar.dma_start(
                st2Tb[:], gout[:, :, F1, :].rearrange("c s p -> p c s"))
            st2T = P.tile([128, 4, 4], F32, tag="st2T")
            nc.scalar.activation(st2T[:], st2Tb[:], AF.Copy)
            D2v = P.tile([128, 4, 4], F32, tag="D2v")
            nc.scalar.activation(D2v[:], st2T[:], AF.Exp, scale=0.2)

            ps_g2 = PS.tile([128, 512], F32, tag="ps")
            nc.tensor.matmul(ps_g2[0:128, 0:RB], ones1b[:], g2row[:])
            g2bc = GB.tile([128, RB], BF16, tag="gb")
            nc.vector.tensor_copy(g2bc[:], ps_g2[0:128, 0:RB])

            W2vf = P.tile([128, 4, 4], F32, tag="W2vf")
            nc.scalar.activation(W2vf[:], st2T[:], AF.Exp, scale=0.8)
            numT2 = PSN.tile([65, 512], F32, tag="numT")
            for jt in range(NJT):
                c4, s4 = jt // 4, jt % 4
                q2 = WK.tile([128, RB], BF16, tag="q")
                F2 = WK.tile([128, RB], BF16, tag="F")
                eng = nc.gpsimd if (jt % 3 == 2 or jt == 7) else nc.vector
                eng.tensor_scalar(q2[:], g2bc[:], W2vf[:, c4, s4:s4 + 1],
                                  D2v[:, c4, s4:s4 + 1], ALU.max, ALU.mult)
                eng.tensor_tensor(F2[:], q2[:], adjT[jt][:], ALU.mult)
                nc.tensor.matmul(numT2[0:65, 0:RB], p2e[:, jt, :],
                                 F2[:], start=(jt == 0), stop=(jt == NJT - 1))

            den2 = P.tile([1, RB], F32, tag="den2")
            nc.scalar.activation(den2[:], numT2[64:65, 0:RB], AF.Copy)
            rec2 = P.tile([1, RB], F32, tag="rec2")
            nc.vector.reciprocal_approx_fast(rec2[:], den2[:])
            rec2b = P.tile([1, RB], BF16, tag="rec2b")
            nc.vector.tensor_copy(rec2b[:], rec2[:])
            ps_r2 = PS.tile([128, 512], F32, tag="ps")
            nc.tensor.matmul(ps_r2[0:64, 0:RB], ones1b[0:1, 0:64], rec2b[:])
            rdb2 = GB.tile([128, RB], BF16, tag="rdb")
            nc.vector.tensor_copy(rdb2[0:64, :], ps_r2[0:64, 0:RB])

            ps_sk2 = PS.tile([128, 512], F32, tag="ps")
            for kt in range(4):
                nc.tensor.matmul(ps_sk2[0:64, 0:RB], ws2Tb[:, kt, :],
                                 houtb[:, kt, :], start=(kt == 0), stop=(kt == 3))

            t2 = WK.tile([F1, RB], F32, tag="t2")
            nc.vector.tensor_mul(t2[:], numT2[0:64, 0:RB], rdb2[0:64, :])
            o2 = WK.tile([F1, RB], F32, tag="o2")
            nc.vector.scalar_tensor_tensor(
                o2[:], t2[:], b2f[:], ps_sk2[0:64, 0:RB], ALU.add, ALU.add)
            nc.sync.dma_start(d_out[:, :], o2[:])

    nc.compile()
    return nc


_NC_CACHE = None


def _get_nc():
    global _NC_CACHE
    if _NC_CACHE is None:
        _NC_CACHE = build_nc()
    return _NC_CACHE


def kernel(x, adj, W1, a_src1, a_tgt1, Wskip1, b1, W2, a_src2, a_tgt2,
           Wskip2, b2):
    x = np.asarray(x, np.float32)
    adj = np.asarray(adj, np.float32)
    nc = _get_nc()
    in_maps = []
    for c in range(NCORES):
        b, r = c // 4, c % 4
        sl = slice(r * RB, (r + 1) * RB)
        in_maps.append({
            "xb": x[b], "xbr": x[b][sl], "adjr": adj[b][sl],
            "w1": np.asarray(W1, np.float32),
            "wskip1": np.asarray(Wskip1, np.float32),
            "asrc1": np.asarray(a_src1, np.float32),
            "atgt1": np.asarray(a_tgt1, np.float32),
            "b1": np.asarray(b1, np.float32),
            "w2": np.asarray(W2, np.float32),
            "wskip2": np.asarray(Wskip2, np.float32),
            "asrc2": np.asarray(a_src2, np.float32),
            "atgt2": np.asarray(a_tgt2, np.float32),
            "b2": np.asarray(b2, np.float32),
        })
    res = run_bass_kernel_spmd(nc, in_maps, core_ids=list(range(NCORES)))
    out = np.empty((BS, N, F1), np.float32)
    for c in range(NCORES):
        b, r = c // 4, c % 4
        out[b, r * RB:(r + 1) * RB, :] = res.results[c]["outT"].T
    return out



# revision 28
# speedup vs baseline: 1.3103x; 1.3103x over previous
"""GAT (2-layer graph attention) Trainium2 Bass kernel, 8-core SPMD.

Sharding: data-parallel over batch (2) x row-blocks (4) -> 8 cores.
Core c handles batch b=c//4, output rows R=[512*(c%4), 512*(c%4+1)).

Key algebra: with z = s_src[i]+s_tgt[j], the GAT edge weight
exp(leaky_relu(z, 0.2)) = max(exp(z), exp(0.2 z)).  Softmax rows are
invariant to a per-row scale, so dividing row i by exp(s_src[i]) gives
unnormalized weights F[j,i] = adj[i,j] * D[j] * max(W[j], g[i]) with
  W[j] = exp(0.8 s_tgt[j]),  D[j] = exp(0.2 s_tgt[j]),  g[i] = exp(-0.8 s_src[i])

Layout strategy (v2):
- Host passes adj column-slices pre-transposed and pre-cast to bf16
  (adj is 0/1 so the cast is exact), plus x^T and all weight transposes,
  so no on-chip transposes/casts of inputs are needed.
- The n x n inner work per (head, j-tile) is either one fused
  scalar_tensor_tensor on GPSIMD (max with W, mult by adjT; D folded
  into the stationary operand) or a tensor_scalar + tensor_tensor pair
  on DVE.  g broadcasts ride the idle SP DMA queue.
- The numerator matmul is i-partitioned: stationary = F chunks
  [128j x 128i], moving = proj rows [128j x 65], accumulating into one
  PSUM bank per head ([128, 4, 65]); column 64 gives the softmax
  denominator, which is then a per-partition scalar in the epilogue
  (no reciprocal-broadcast matmuls).

Layer-1 outputs are exchanged within each batch group of 4 cores via a
single AllGather of (proj2^T | s_tgt2) so layer 2 stays row-local.
"""

import os
import sys

for _p in ("/opt/trn_rl_repo", "/root/.axon_site/_ro/trn_rl_repo"):
    if os.path.isdir(_p) and _p not in sys.path:
        sys.path.insert(0, _p)

import numpy as np

import concourse.bass as bass
import concourse.bacc as bacc
import concourse.mybir as mybir
from concourse import tile
from concourse.bass_utils import run_bass_kernel_spmd

F32 = mybir.dt.float32
BF16 = mybir.dt.bfloat16
AF = mybir.ActivationFunctionType
ALU = mybir.AluOpType

BS, N, FIN = 2, 2048, 128
H1, F1 = 8, 64
RB = 512          # row block per core
NJT = N // 128    # 16 j-tiles
NIT = RB // 128   # 4 i-tiles in the row block
NCORES = 8
GROUPS = [[0, 1, 2, 3], [4, 5, 6, 7]]


def _mode_a(h, jp):
    # 'a' pairs: TSP+TT both on DVE; others: TSP on DVE, TT on Pool
    return (h * 8 + jp) % 10 < 3


def build_nc():
    nc = bacc.Bacc("TRN2", target_bir_lowering=False, debug=False,
                   num_devices=NCORES)

    # ---- per-core DRAM I/O (host pre-transposes / pre-casts) ----
    d_xT = nc.declare_dram_parameter("xT", [FIN, N], F32, isOutput=False)
    d_xTb = nc.declare_dram_parameter("xTb", [FIN, N], BF16, isOutput=False)
    d_xrT = nc.declare_dram_parameter("xrT", [FIN, RB], F32, isOutput=False)
    d_xrTb = nc.declare_dram_parameter("xrTb", [FIN, RB], BF16, isOutput=False)
    d_adjT = nc.declare_dram_parameter("adjT", [N, RB], BF16, isOutput=False)
    d_w1n = nc.declare_dram_parameter("w1", [H1 * F1, FIN], F32, isOutput=False)
    d_w1Tb = nc.declare_dram_parameter("w1Tb", [FIN, H1 * F1], BF16,
                                       isOutput=False)
    d_ws1Tb = nc.declare_dram_parameter("ws1Tb", [FIN, H1 * F1], BF16,
                                        isOutput=False)
    d_as1 = nc.declare_dram_parameter("asrc1", [H1, F1], F32, isOutput=False)
    d_at1 = nc.declare_dram_parameter("atgt1", [H1, F1], F32, isOutput=False)
    d_b1r = nc.declare_dram_parameter("b1r", [1, H1 * F1], F32, isOutput=False)
    d_w2 = nc.declare_dram_parameter("w2", [F1, H1 * F1], F32, isOutput=False)
    d_w2Tb = nc.declare_dram_parameter("w2Tb", [H1 * F1, F1], BF16,
                                       isOutput=False)
    d_ws2Tb = nc.declare_dram_parameter("ws2Tb", [H1 * F1, F1], BF16,
                                        isOutput=False)
    d_as2 = nc.declare_dram_parameter("asrc2", [1, F1], F32, isOutput=False)
    d_at2 = nc.declare_dram_parameter("atgt2", [1, F1], F32, isOutput=False)
    d_b2 = nc.declare_dram_parameter("b2", [F1], F32, isOutput=False)
    # output: natural row-block [512, 64]
    d_out = nc.declare_dram_parameter("outN", [RB, F1], F32, isOutput=True)

    with tile.TileContext(nc) as tc:
        with (
            tc.tile_pool(name="persist", bufs=1) as P,
            tc.tile_pool(name="work", bufs=8) as WK,
            tc.tile_pool(name="gbp", bufs=3) as GB,
            tc.tile_pool(name="ps", bufs=3, space="PSUM") as PS,
            tc.tile_pool(name="psnum", bufs=3, space="PSUM") as PSN,
            tc.tile_pool(name="pst", bufs=2, space="PSUM") as PST,
            tc.tile_pool(name="dram", bufs=1, space="DRAM") as DR,
        ):
            # ============ loads (all natural-layout now) ====================
            # SP queue: xT first (scores path), then xTb, then adjT half
            xT = P.tile([128, N], F32, tag="xT")
            nc.sync.dma_start(xT[:], d_xT[:, :])
            xTb = P.tile([128, N], BF16, tag="xTb")
            nc.sync.dma_start(xTb[:], d_xTb[:, :])
            # Act queue: adjT other half + xr + small weights
            adjTb = P.tile([128, NJT, RB], BF16, tag="adjTb")
            nc.sync.dma_start(
                adjTb[:, 0:NJT // 2, :],
                d_adjT[0:N // 2, :].rearrange("(t p) i -> p t i", p=128))
            nc.sync.dma_start(
                adjTb[:, NJT // 2:NJT, :],
                d_adjT[N // 2:N, :].rearrange("(t p) i -> p t i", p=128))
            xrT = P.tile([128, RB], F32, tag="xrT")
            nc.scalar.dma_start(xrT[:], d_xrT[:, :])
            xrTb = P.tile([128, RB], BF16, tag="xrTb")
            nc.scalar.dma_start(xrTb[:], d_xrTb[:, :])
            # Pool queue: a-vectors + weights needed early for scores/proj1
            a1sT = P.tile([128, H1], F32, tag="a1sT")
            nc.gpsimd.dma_start(a1sT[0:F1, :], d_as1.rearrange("h f -> f h"))
            nc.gpsimd.dma_start(a1sT[F1:2 * F1, :], d_as1.rearrange("h f -> f h"))
            a1tT = P.tile([128, H1], F32, tag="a1tT")
            nc.gpsimd.dma_start(a1tT[0:F1, :], d_at1.rearrange("h f -> f h"))
            nc.gpsimd.dma_start(a1tT[F1:2 * F1, :], d_at1.rearrange("h f -> f h"))
            w1n = P.tile([128, 4, FIN], F32, tag="w1n")
            nc.gpsimd.dma_start(w1n[:], d_w1n.rearrange("(k p) c -> p k c", p=128))
            w1Tb = P.tile([128, H1 * F1], BF16, tag="w1Tb")
            nc.gpsimd.dma_start(w1Tb[:], d_w1Tb[:, :])
            ws1Tb = P.tile([128, H1 * F1], BF16, tag="ws1Tb")
            nc.gpsimd.dma_start(ws1Tb[:], d_ws1Tb[:, :])
            b1rb = P.tile([1, H1 * F1], BF16, tag="b1rb")
            nc.gpsimd.dma_start(b1rb[:], d_b1r[:, :])
            # L2 weights ride the SP queue later (needed only at L2 time)
            w2n = P.tile([F1, H1 * F1], F32, tag="w2n")
            nc.sync.dma_start(w2n[:], d_w2[:, :])
            w2Tb = P.tile([128, 4, F1], BF16, tag="w2Tb")
            nc.sync.dma_start(w2Tb[:], d_w2Tb.rearrange("(k p) f -> p k f", p=128))
            ws2Tb = P.tile([128, 4, F1], BF16, tag="ws2Tb")
            nc.sync.dma_start(ws2Tb[:], d_ws2Tb.rearrange("(k p) f -> p k f", p=128))
            a2p = P.tile([F1, 2], F32, tag="a2p")
            nc.sync.dma_start(a2p[:, 0:1], d_as2.rearrange("o f -> f o"))
            nc.sync.dma_start(a2p[:, 1:2], d_at2.rearrange("o f -> f o"))
            b2row = P.tile([1, F1], F32, tag="b2row")
            nc.sync.dma_start(b2row[:], d_b2.ap().rearrange("(o f) -> o f", o=1))

            ones1b = P.tile([1, 128], BF16, tag="ones1b")
            nc.vector.memset(ones1b[:], 1.0)
            # identity (bf16) for PE transposes of h
            onesq = P.tile([128, 128], BF16, tag="onesq")
            nc.vector.memset(onesq[:], 1.0)
            ident = P.tile([128, 128], BF16, tag="ident")
            nc.gpsimd.affine_select(ident[:], onesq[:], [[-1, 128]],
                                    ALU.is_equal, 0.0, base=0,
                                    channel_multiplier=1)

            # ============ scores (exact fp32) ===============================
            # w1tilde [c=128, 16]: col h = W1_h^T a_src1[h], col 8+h tgt
            ps_wt = PS.tile([128, 512], F32, tag="ps")
            for h in range(H1):
                kt, pr = (h * F1) // 128, (h * F1) % 128
                w1slc = w1n[pr:pr + F1, kt, :]
                nc.tensor.matmul(ps_wt[0:128, h:h + 1], w1slc,
                                 a1sT[pr:pr + F1, h:h + 1])
                nc.tensor.matmul(ps_wt[0:128, 8 + h:9 + h], w1slc,
                                 a1tT[pr:pr + F1, h:h + 1])
            w1t = P.tile([128, 16], F32, tag="w1t")
            nc.vector.tensor_copy(w1t[:], ps_wt[0:128, 0:16])

            # S1T [j(128 x 16 chunks), 16] = x @ w1tilde
            ps_s1t = PS.tile([128, 512], F32, tag="ps")
            for jc in range(NJT):
                nc.tensor.matmul(ps_s1t[0:128, jc * 16:(jc + 1) * 16],
                                 xT[:, jc * 128:(jc + 1) * 128], w1t[:])
            s1T = P.tile([128, NJT * 16], F32, tag="s1T")
            nc.vector.tensor_copy(s1T[:], ps_s1t[0:128, 0:NJT * 16])
            Wvf = P.tile([128, NJT * 16], F32, tag="Wvf")
            nc.scalar.activation(Wvf[:], s1T[:], AF.Exp, scale=0.8)
            Dvf = P.tile([128, NJT * 16], F32, tag="Dvf")
            nc.scalar.activation(Dvf[:], s1T[:], AF.Exp, scale=0.2)

            # s_src rows for our block -> g (bf16) [16, 512]
            ps_s1r = PS.tile([128, 512], F32, tag="ps")
            nc.tensor.matmul(ps_s1r[0:16, 0:RB], w1t[:], xrT[:])
            g1b = P.tile([16, RB], BF16, tag="g1b")
            nc.scalar.activation(g1b[:], ps_s1r[0:16, 0:RB], AF.Exp, scale=-0.8)

            # selector tiles sel_h [16, 128] (row h ones) for g broadcasts
            ones16 = P.tile([16, 128], BF16, tag="ones16")
            nc.vector.memset(ones16[:], 1.0)
            sel = P.tile([16, H1 * 128], BF16, tag="sel")
            for h in range(H1):
                nc.gpsimd.affine_select(sel[:, h * 128:(h + 1) * 128],
                                        ones16[:], [[0, 128]], ALU.is_equal,
                                        0.0, base=-h, channel_multiplier=1)

            # g broadcasts: PE selector matmul + copy out of PSUM
            gbhs = []
            for h in range(H1):
                ps_g = PS.tile([128, 512], F32, tag="ps")
                nc.tensor.matmul(ps_g[0:128, 0:RB],
                                 sel[:, h * 128:(h + 1) * 128], g1b[:])
                gbh = P.tile([128, RB], BF16, tag=f"gbh{h}", name=f"gbh{h}")
                nc.scalar.activation(gbh[:], ps_g[0:128, 0:RB], AF.Copy)
                gbhs.append(gbh)

            # ============ proj1 -> p1e (+ ones col) =========================
            p1e = P.tile([128, NJT, 8, 66], BF16, tag="p1e")
            nc.vector.memset(p1e[:, :, :, 64:65], 1.0)
            for jt in range(NJT):
                ps_p = PS.tile([128, 512], F32, tag="ps")
                nc.tensor.matmul(ps_p[0:128, 0:512],
                                 xTb[:, jt * 128:(jt + 1) * 128], w1Tb[:])
                dst = p1e[:, jt, :, 0:64]
                src = ps_p[0:128, 0:512].rearrange("p (h q) -> p h q", q=64)
                nc.scalar.activation(dst, src, AF.Copy)

            # ============ layer-1 skip (natural layout, + bias) =============
            # skipsb [128 i, 4 ich, 512 hf] bf16
            skipsb = P.tile([128, 4, H1 * F1], BF16, tag="skipsb")
            for ich in range(NIT):
                ps_sk = PS.tile([128, 512], F32, tag="ps")
                nc.tensor.matmul(ps_sk[0:128, 0:512],
                                 xrTb[:, ich * 128:(ich + 1) * 128], ws1Tb[:],
                                 start=True, stop=False)
                nc.tensor.matmul(ps_sk[0:128, 0:512], ones1b[0:1, 0:128],
                                 b1rb[:], start=False, stop=True)
                nc.scalar.activation(skipsb[:, ich, :], ps_sk[0:128, 0:512],
                                     AF.Copy)

            ps_w2 = PS.tile([128, 512], F32, tag="ps")
            for kt in range(4):
                nc.tensor.matmul(ps_w2[0:128, kt * 2:kt * 2 + 2],
                                 w2n[:, kt * 128:(kt + 1) * 128], a2p[:],
                                 start=True, stop=True)
            w2tb = P.tile([128, 8], BF16, tag="w2tb")
            nc.vector.tensor_copy(w2tb[:], ps_w2[0:128, 0:8])

            # ============ layer-1 head loop (i-part numerator) ==============
            hnat = P.tile([128, 4, H1 * F1], BF16, tag="hnat")
            houtb = P.tile([128, 4, RB], BF16, tag="houtb")
            for h in range(H1):
                ps_h = PSN.tile([128, 4, 65], F32, tag="psn")
                for jp in range(NJT // 2):
                    q = WK.tile([128, 2, RB], BF16, tag="q")
                    for l in range(2):
                        jt = 2 * jp + l
                        col = slice(jt * 16 + 8 + h, jt * 16 + 9 + h)
                        nc.vector.tensor_scalar(q[:, l, :], gbhs[h][:],
                                                Wvf[:, col], Dvf[:, col],
                                                ALU.max, ALU.mult)
                    Ft = WK.tile([128, 2, RB], BF16, tag="F")
                    eng = nc.vector if _mode_a(h, jp) else nc.gpsimd
                    eng.tensor_tensor(Ft[:], q[:],
                                      adjTb[:, 2 * jp:2 * jp + 2, :], ALU.mult)
                    for l in range(2):
                        jt = 2 * jp + l
                        mv = p1e[:, jt, h, 0:65]
                        for ich in range(NIT):
                            nc.tensor.matmul(
                                ps_h[:, ich, 0:65],
                                Ft[:, l, ich * 128:(ich + 1) * 128], mv,
                                start=(jt == 0 and ich == 0),
                                stop=(jt == NJT - 1 and ich == NIT - 1))

                # epilogue: h_nat[:, :, h*64:(h+1)*64] = elu(num/den + skip)
                rec = WK.tile([128, 4], F32, tag="rec")
                nc.vector.reciprocal_approx_fast(
                    rec[:], ps_h[:, :, 64:65].rearrange("p a o -> p (a o)"))
                u1 = WK.tile([128, 4, 64], BF16, tag="u1")
                nc.vector.tensor_tensor(
                    u1[:], ps_h[:, :, 0:64],
                    rec[:].unsqueeze(2).to_broadcast((128, 4, 64)),
                    ALU.mult)
                u = WK.tile([128, 4, 64], BF16, tag="u")
                nc.gpsimd.tensor_tensor(
                    u[:], u1[:],
                    skipsb[:].rearrange("p a (g f) -> p a g f", f=64)[:, :, h, :],
                    ALU.add)
                m0 = WK.tile([128, 4, 64], BF16, tag="m0")
                nc.gpsimd.tensor_scalar(m0[:], u[:], 0.0, 1.0, ALU.min, ALU.mult)
                e = WK.tile([128, 4, 64], BF16, tag="e")
                nc.scalar.activation(e[:], m0[:], AF.Exp)
                nc.vector.scalar_tensor_tensor(
                    hnat[:].rearrange("p a (g f) -> p a g f", f=64)[:, :, h, :],
                    e[:], -1.0, u[:], ALU.add, ALU.max)

                # hnat [128 i, 4 ich, hf] -> houtb (h^T) [128 hf, kt, 512 i],
                # transposed per head-pair as soon as both heads are done
                if h % 2 == 1:
                    kt = h // 2
                    ps_t = PST.tile([128, 512], BF16, tag="pst")
                    for ich in range(NIT):
                        nc.tensor.transpose(
                            ps_t[:, ich * 128:(ich + 1) * 128],
                            hnat[:, ich, kt * 128:(kt + 1) * 128], ident[:])
                    nc.scalar.activation(houtb[:, kt, :], ps_t[:], AF.Copy)

            # ============ layer-2 local pieces ==============================
            # S2: s_src2 -> psum row 0, s_tgt2 -> psum row 32
            ps_s2 = PS.tile([128, 512], F32, tag="ps")
            for kt in range(4):
                nc.tensor.matmul(ps_s2[0:1, 0:RB], w2tb[:, kt * 2:kt * 2 + 1],
                                 houtb[:, kt, :], start=(kt == 0), stop=(kt == 3))
            for kt in range(4):
                nc.tensor.matmul(ps_s2[32:33, 0:RB], w2tb[:, kt * 2 + 1:kt * 2 + 2],
                                 houtb[:, kt, :], start=(kt == 0), stop=(kt == 3))
            g2row = P.tile([1, RB], BF16, tag="g2row")
            nc.scalar.activation(g2row[:], ps_s2[0:1, 0:RB], AF.Exp, scale=-0.8)
            stg2 = P.tile([1, RB], F32, tag="stg2")
            nc.scalar.activation(stg2[:], ps_s2[32:33, 0:RB], AF.Copy)

            # proj2^T local [64, 512] in bf16 for the gather
            ps_p2 = PS.tile([128, 512], F32, tag="ps")
            for kt in range(4):
                nc.tensor.matmul(ps_p2[0:64, 0:RB], w2Tb[:, kt, :],
                                 houtb[:, kt, :], start=(kt == 0), stop=(kt == 3))
            p2Tb = P.tile([F1, RB], BF16, tag="p2Tb")
            nc.scalar.activation(p2Tb[:], ps_p2[0:64, 0:RB], AF.Copy)

            # ============ layer-2 epilogue (natural) ========================
            # skip2 natural [128 i, 64] per ich, with b2 folded in via a
            # rank-1 bias matmul
            b2rowb = P.tile([1, F1], BF16, tag="b2rowb")
            nc.vector.tensor_copy(b2rowb[:], b2row[:])
            sk2sb = P.tile([128, 4, F1], F32, tag="sk2sb")
            for ich in range(NIT):
                pssk = PS.tile([128, 512], F32, tag="ps")
                for kt in range(4):
                    nc.tensor.matmul(pssk[0:128, 0:F1],
                                     houtb[:, kt, ich * 128:(ich + 1) * 128],
                                     ws2Tb[:, kt, :],
                                     start=(kt == 0), stop=False)
                nc.tensor.matmul(pssk[0:128, 0:F1], ones1b[0:1, 0:128],
                                 b2rowb[:], start=False, stop=True)
                nc.scalar.activation(sk2sb[:, ich, :], pssk[0:128, 0:F1],
                                     AF.Copy)

            ps_g2 = PS.tile([128, 512], F32, tag="ps")
            nc.tensor.matmul(ps_g2[0:128, 0:RB], ones1b[:], g2row[:])
            g2bc = GB.tile([128, RB], BF16, tag="gb")
            nc.vector.tensor_copy(g2bc[:], ps_g2[0:128, 0:RB])

            # ============ AllGather within batch group ======================
            gin = DR.tile([4, F1 + 1, 128], BF16)
            nc.sync.dma_start(
                gin[:, 0:F1, :].rearrange("s f p -> f s p"),
                p2Tb[:].rearrange("f (s p) -> f s p", p=128))
            stg2b = P.tile([1, RB], BF16, tag="stg2b")
            nc.vector.tensor_copy(stg2b[:], stg2[:])
            nc.sync.dma_start(
                gin[:, F1:F1 + 1, :].rearrange("s o p -> o s p"),
                stg2b[:].rearrange("o (s p) -> o s p", p=128))
            gout = DR.tile([4, 4, F1 + 1, 128], BF16)
            nc.gpsimd.collective_compute(
                "AllGather", ALU.bypass, replica_groups=GROUPS,
                ins=[gin.opt()], outs=[gout.opt()])

            # ============ layer-2 attention =================================
            p2e = P.tile([128, NJT, F1 + 1], BF16, tag="p2e")
            nc.sync.dma_start(
                p2e[:],
                gout.rearrange("c s f p -> p (c s) f"))
            # denominator column must be 1 (on Pool: keeps DVE free for TSPs)
            nc.gpsimd.memset(p2e[:, :, F1:F1 + 1], 1.0)
            st2Tb = P.tile([128, 4, 4], BF16, tag="st2Tb")
            nc.scalar.dma_start(
                st2Tb[:], gout[:, :, F1, :].rearrange("c s p -> p c s"))
            D2v = P.tile([128, 4, 4], F32, tag="D2v")
            nc.scalar.activation(D2v[:], st2Tb[:], AF.Exp, scale=0.2)
            W2vf = P.tile([128, 4, 4], F32, tag="W2vf")
            nc.scalar.activation(W2vf[:], st2Tb[:], AF.Exp, scale=0.8)

            ps_h2 = PSN.tile([128, 4, 65], F32, tag="psn")
            for jp in range(NJT // 2):
                q2 = WK.tile([128, 2, RB], BF16, tag="q")
                for l in range(2):
                    jt = 2 * jp + l
                    c4, s4 = jt // 4, jt % 4
                    nc.vector.tensor_scalar(q2[:, l, :], g2bc[:],
                                            W2vf[:, c4, s4:s4 + 1],
                                            D2v[:, c4, s4:s4 + 1],
                                            ALU.max, ALU.mult)
                F2 = WK.tile([128, 2, RB], BF16, tag="F")
                eng = nc.gpsimd if jp % 8 < 5 else nc.vector
                eng.tensor_tensor(F2[:], q2[:],
                                  adjTb[:, 2 * jp:2 * jp + 2, :], ALU.mult)
                for l in range(2):
                    jt = 2 * jp + l
                    mv = p2e[:, jt, :]
                    for ich in range(NIT):
                        nc.tensor.matmul(
                            ps_h2[:, ich, 0:65],
                            F2[:, l, ich * 128:(ich + 1) * 128], mv,
                            start=(jt == 0 and ich == 0),
                            stop=(jt == NJT - 1 and ich == NIT - 1))

            rec2 = WK.tile([128, 4], F32, tag="rec")
            nc.vector.reciprocal_approx_fast(
                rec2[:], ps_h2[:, :, 64:65].rearrange("p a o -> p (a o)"))
            onat = WK.tile([128, 4, F1], F32, tag="onat")
            t2 = WK.tile([128, 4, F1], F32, tag="t2")
            nc.vector.tensor_tensor(
                t2[:], ps_h2[:, :, 0:64],
                rec2[:].unsqueeze(2).to_broadcast((128, 4, F1)), ALU.mult)
            nc.vector.tensor_tensor(onat[:], t2[:], sk2sb[:], ALU.add)
            nc.sync.dma_start(
                d_out.rearrange("(a p) f -> p a f", p=128), onat[:])

    nc.compile()
    return nc


_NC_CACHE = None


def _get_nc():
    global _NC_CACHE
    if _NC_CACHE is None:
        _NC_CACHE = build_nc()
    return _NC_CACHE


def make_in_maps(x, adj, W1, a_src1, a_tgt1, Wskip1, b1, W2, a_src2, a_tgt2,
                 Wskip2, b2):
    import ml_dtypes
    bf16 = ml_dtypes.bfloat16
    x = np.asarray(x, np.float32)
    adj = np.asarray(adj, np.float32)
    W1 = np.asarray(W1, np.float32)
    W2 = np.asarray(W2, np.float32)
    Wskip1 = np.asarray(Wskip1, np.float32)
    Wskip2 = np.asarray(Wskip2, np.float32)
    in_maps = []
    for c in range(NCORES):
        b, r = c // 4, c % 4
        sl = slice(r * RB, (r + 1) * RB)
        xTf = np.ascontiguousarray(x[b].T)
        xrTf = np.ascontiguousarray(x[b][sl].T)
        in_maps.append({
            "xT": xTf, "xTb": xTf.astype(bf16),
            "xrT": xrTf, "xrTb": xrTf.astype(bf16),
            "adjT": np.ascontiguousarray(adj[b][sl].T).astype(bf16),
            "w1": W1,
            "w1Tb": np.ascontiguousarray(W1.T).astype(bf16),
            "ws1Tb": np.ascontiguousarray(Wskip1.T).astype(bf16),
            "asrc1": np.asarray(a_src1, np.float32),
            "atgt1": np.asarray(a_tgt1, np.float32),
            "b1r": np.asarray(b1, np.float32).reshape(1, -1),
            "w2": W2,
            "w2Tb": np.ascontiguousarray(W2.T).astype(bf16),
            "ws2Tb": np.ascontiguousarray(Wskip2.T).astype(bf16),
            "asrc2": np.asarray(a_src2, np.float32),
            "atgt2": np.asarray(a_tgt2, np.float32),
            "b2": np.asarray(b2, np.float32),
        })
    return in_maps


def kernel(x, adj, W1, a_src1, a_tgt1, Wskip1, b1, W2, a_src2, a_tgt2,
           Wskip2, b2):
    nc = _get_nc()
    in_maps = make_in_maps(x, adj, W1, a_src1, a_tgt1, Wskip1, b1, W2,
                           a_src2, a_tgt2, Wskip2, b2)
    res = run_bass_kernel_spmd(nc, in_maps, core_ids=list(range(NCORES)))
    out = np.empty((BS, N, F1), np.float32)
    for c in range(NCORES):
        b, r = c // 4, c % 4
        out[b, r * RB:(r + 1) * RB, :] = res.results[c]["outN"]
    return out


# revision 35
# speedup vs baseline: 1.3155x; 1.0040x over previous
"""GAT (2-layer graph attention) Trainium2 Bass kernel, 8-core SPMD.

Sharding: data-parallel over batch (2) x row-blocks (4) -> 8 cores.
Core c handles batch b=c//4, output rows R=[512*(c%4), 512*(c%4+1)).

Key algebra: with z = s_src[i]+s_tgt[j], the GAT edge weight
exp(leaky_relu(z, 0.2)) = max(exp(z), exp(0.2 z)).  Softmax rows are
invariant to a per-row scale, so dividing row i by exp(s_src[i]) gives
unnormalized weights F[j,i] = adj[i,j] * D[j] * max(W[j], g[i]) with
  W[j] = exp(0.8 s_tgt[j]),  D[j] = exp(0.2 s_tgt[j]),  g[i] = exp(-0.8 s_src[i])

Layout strategy (v2):
- Host passes adj column-slices pre-transposed and pre-cast to bf16
  (adj is 0/1 so the cast is exact), plus x^T and all weight transposes,
  so no on-chip transposes/casts of inputs are needed.
- The n x n inner work per (head, j-tile) is either one fused
  scalar_tensor_tensor on GPSIMD (max with W, mult by adjT; D folded
  into the stationary operand) or a tensor_scalar + tensor_tensor pair
  on DVE.  g broadcasts ride the idle SP DMA queue.
- The numerator matmul is i-partitioned: stationary = F chunks
  [128j x 128i], moving = proj rows [128j x 65], accumulating into one
  PSUM bank per head ([128, 4, 65]); column 64 gives the softmax
  denominator, which is then a per-partition scalar in the epilogue
  (no reciprocal-broadcast matmuls).

Layer-1 outputs are exchanged within each batch group of 4 cores via a
single AllGather of (proj2^T | s_tgt2) so layer 2 stays row-local.
"""

import os
import sys

for _p in ("/opt/trn_rl_repo", "/root/.axon_site/_ro/trn_rl_repo"):
    if os.path.isdir(_p) and _p not in sys.path:
        sys.path.insert(0, _p)

import numpy as np

import concourse.bass as bass
import concourse.bacc as bacc
import concourse.mybir as mybir
from concourse import tile
from concourse.bass_utils import run_bass_kernel_spmd

F32 = mybir.dt.float32
BF16 = mybir.dt.bfloat16
AF = mybir.ActivationFunctionType
ALU = mybir.AluOpType

BS, N, FIN = 2, 2048, 128
H1, F1 = 8, 64
RB = 512          # row block per core
NJT = N // 128    # 16 j-tiles
NIT = RB // 128   # 4 i-tiles in the row block
NCORES = 8
GROUPS = [[0, 1, 2, 3], [4, 5, 6, 7]]


def _mode_a(h, jp):
    # 'a' pairs: TSP+TT both on DVE; others: TSP on DVE, TT on Pool
    return (h * 8 + jp) % 10 < 3


def build_nc():
    nc = bacc.Bacc("TRN2", target_bir_lowering=False, debug=False,
                   num_devices=NCORES)

    # ---- per-core DRAM I/O (host pre-transposes / pre-casts) ----
    d_xT = nc.declare_dram_parameter("xT", [FIN, N], F32, isOutput=False)
    d_xTb = nc.declare_dram_parameter("xTb", [FIN, N], BF16, isOutput=False)
    d_xrT = nc.declare_dram_parameter("xrT", [FIN, RB], F32, isOutput=False)
    d_xrTb = nc.declare_dram_parameter("xrTb", [FIN, RB], BF16, isOutput=False)
    d_adjT = nc.declare_dram_parameter("adjT", [N, RB], BF16, isOutput=False)
    d_w1n = nc.declare_dram_parameter("w1", [H1 * F1, FIN], F32, isOutput=False)
    d_w1Tb = nc.declare_dram_parameter("w1Tb", [FIN, H1 * F1], BF16,
                                       isOutput=False)
    d_ws1Tb = nc.declare_dram_parameter("ws1Tb", [FIN, H1 * F1], BF16,
                                        isOutput=False)
    d_as1 = nc.declare_dram_parameter("asrc1", [H1, F1], F32, isOutput=False)
    d_at1 = nc.declare_dram_parameter("atgt1", [H1, F1], F32, isOutput=False)
    d_b1r = nc.declare_dram_parameter("b1r", [1, H1 * F1], F32, isOutput=False)
    d_w2 = nc.declare_dram_parameter("w2", [F1, H1 * F1], F32, isOutput=False)
    d_w2Tb = nc.declare_dram_parameter("w2Tb", [H1 * F1, F1], BF16,
                                       isOutput=False)
    d_ws2Tb = nc.declare_dram_parameter("ws2Tb", [H1 * F1, F1], BF16,
                                        isOutput=False)
    d_as2 = nc.declare_dram_parameter("asrc2", [1, F1], F32, isOutput=False)
    d_at2 = nc.declare_dram_parameter("atgt2", [1, F1], F32, isOutput=False)
    d_b2 = nc.declare_dram_parameter("b2", [F1], F32, isOutput=False)
    # output: natural row-block [512, 64]
    d_out = nc.declare_dram_parameter("outN", [RB, F1], F32, isOutput=True)

    with tile.TileContext(nc) as tc:
        with (
            tc.tile_pool(name="persist", bufs=1) as P,
            tc.tile_pool(name="work", bufs=8) as WK,
            tc.tile_pool(name="gbp", bufs=3) as GB,
            tc.tile_pool(name="ps", bufs=3, space="PSUM") as PS,
            tc.tile_pool(name="psnum", bufs=4, space="PSUM") as PSN,
            tc.tile_pool(name="pst", bufs=1, space="PSUM") as PST,
            tc.tile_pool(name="dram", bufs=1, space="DRAM") as DR,
        ):
            # ============ loads (all natural-layout now) ====================
            # SP queue: xT first (scores path), then xTb, then adjT half
            xT = P.tile([128, N], F32, tag="xT")
            nc.sync.dma_start(xT[:], d_xT[:, :])
            xTb = P.tile([128, N], BF16, tag="xTb")
            nc.sync.dma_start(xTb[:], d_xTb[:, :])
            # Act queue: adjT other half + xr + small weights
            adjTb = P.tile([128, NJT, RB], BF16, tag="adjTb")
            nc.sync.dma_start(
                adjTb[:, 0:NJT // 2, :],
                d_adjT[0:N // 2, :].rearrange("(t p) i -> p t i", p=128))
            nc.sync.dma_start(
                adjTb[:, NJT // 2:NJT, :],
                d_adjT[N // 2:N, :].rearrange("(t p) i -> p t i", p=128))
            xrT = P.tile([128, RB], F32, tag="xrT")
            nc.scalar.dma_start(xrT[:], d_xrT[:, :])
            xrTb = P.tile([128, RB], BF16, tag="xrTb")
            nc.scalar.dma_start(xrTb[:], d_xrTb[:, :])
            # Pool queue: a-vectors + weights needed early for scores/proj1
            a1sT = P.tile([128, H1], F32, tag="a1sT")
            nc.gpsimd.dma_start(a1sT[0:F1, :], d_as1.rearrange("h f -> f h"))
            nc.gpsimd.dma_start(a1sT[F1:2 * F1, :], d_as1.rearrange("h f -> f h"))
            a1tT = P.tile([128, H1], F32, tag="a1tT")
            nc.gpsimd.dma_start(a1tT[0:F1, :], d_at1.rearrange("h f -> f h"))
            nc.gpsimd.dma_start(a1tT[F1:2 * F1, :], d_at1.rearrange("h f -> f h"))
            w1n = P.tile([128, 4, FIN], F32, tag="w1n")
            nc.gpsimd.dma_start(w1n[:], d_w1n.rearrange("(k p) c -> p k c", p=128))
            w1Tb = P.tile([128, H1 * F1], BF16, tag="w1Tb")
            nc.gpsimd.dma_start(w1Tb[:], d_w1Tb[:, :])
            ws1Tb = P.tile([128, H1 * F1], BF16, tag="ws1Tb")
            nc.gpsimd.dma_start(ws1Tb[:], d_ws1Tb[:, :])
            b1rb = P.tile([1, H1 * F1], BF16, tag="b1rb")
            nc.gpsimd.dma_start(b1rb[:], d_b1r[:, :])
            # L2 weights ride the SP queue later (needed only at L2 time)
            w2n = P.tile([F1, H1 * F1], F32, tag="w2n")
            nc.sync.dma_start(w2n[:], d_w2[:, :])
            w2Tb = P.tile([128, 4, F1], BF16, tag="w2Tb")
            nc.sync.dma_start(w2Tb[:], d_w2Tb.rearrange("(k p) f -> p k f", p=128))
            ws2Tb = P.tile([128, 4, F1], BF16, tag="ws2Tb")
            nc.sync.dma_start(ws2Tb[:], d_ws2Tb.rearrange("(k p) f -> p k f", p=128))
            a2p = P.tile([F1, 2], F32, tag="a2p")
            nc.sync.dma_start(a2p[:, 0:1], d_as2.rearrange("o f -> f o"))
            nc.sync.dma_start(a2p[:, 1:2], d_at2.rearrange("o f -> f o"))
            b2row = P.tile([1, F1], F32, tag="b2row")
            nc.sync.dma_start(b2row[:], d_b2.ap().rearrange("(o f) -> o f", o=1))

            ones1b = P.tile([1, 128], BF16, tag="ones1b")
            nc.vector.memset(ones1b[:], 1.0)
            # identity (bf16) for PE transposes of h
            onesq = P.tile([128, 128], BF16, tag="onesq")
            nc.vector.memset(onesq[:], 1.0)
            ident = P.tile([128, 128], BF16, tag="ident")
            nc.gpsimd.affine_select(ident[:], onesq[:], [[-1, 128]],
                                    ALU.is_equal, 0.0, base=0,
                                    channel_multiplier=1)

            # ============ scores (exact fp32) ===============================
            # w1tilde [c=128, 16]: col h = W1_h^T a_src1[h], col 8+h tgt
            ps_wt = PS.tile([128, 512], F32, tag="ps")
            for h in range(H1):
                kt, pr = (h * F1) // 128, (h * F1) % 128
                w1slc = w1n[pr:pr + F1, kt, :]
                nc.tensor.matmul(ps_wt[0:128, h:h + 1], w1slc,
                                 a1sT[pr:pr + F1, h:h + 1])
                nc.tensor.matmul(ps_wt[0:128, 8 + h:9 + h], w1slc,
                                 a1tT[pr:pr + F1, h:h + 1])
            w1t = P.tile([128, 16], F32, tag="w1t")
            nc.vector.tensor_copy(w1t[:], ps_wt[0:128, 0:16])

            # S1T [j(128 x 16 chunks), 16] = x @ w1tilde
            ps_s1t = PS.tile([128, 512], F32, tag="ps")
            for jc in range(NJT):
                nc.tensor.matmul(ps_s1t[0:128, jc * 16:(jc + 1) * 16],
                                 xT[:, jc * 128:(jc + 1) * 128], w1t[:])
            s1T = P.tile([128, NJT * 16], F32, tag="s1T")
            nc.vector.tensor_copy(s1T[:], ps_s1t[0:128, 0:NJT * 16])
            Wvf = P.tile([128, NJT * 16], F32, tag="Wvf")
            nc.scalar.activation(Wvf[:], s1T[:], AF.Exp, scale=0.8)
            Dvf = P.tile([128, NJT * 16], F32, tag="Dvf")
            nc.scalar.activation(Dvf[:], s1T[:], AF.Exp, scale=0.2)

            # s_src rows for our block -> g (bf16) [16, 512]
            ps_s1r = PS.tile([128, 512], F32, tag="ps")
            nc.tensor.matmul(ps_s1r[0:16, 0:RB], w1t[:], xrT[:])
            g1b = P.tile([16, RB], BF16, tag="g1b")
            nc.scalar.activation(g1b[:], ps_s1r[0:16, 0:RB], AF.Exp, scale=-0.8)

            # selector tiles sel_h [16, 128] (row h ones) for g broadcasts
            ones16 = P.tile([16, 128], BF16, tag="ones16")
            nc.vector.memset(ones16[:], 1.0)
            sel = P.tile([16, H1 * 128], BF16, tag="sel")
            for h in range(H1):
                nc.gpsimd.affine_select(sel[:, h * 128:(h + 1) * 128],
                                        ones16[:], [[0, 128]], ALU.is_equal,
                                        0.0, base=-h, channel_multiplier=1)

            # g broadcasts: PE selector matmul + copy out of PSUM
            gbhs = []
            for h in range(H1):
                ps_g = PS.tile([128, 512], F32, tag="ps")
                nc.tensor.matmul(ps_g[0:128, 0:RB],
                                 sel[:, h * 128:(h + 1) * 128], g1b[:])
                gbh = P.tile([128, RB], BF16, tag=f"gbh{h}", name=f"gbh{h}")
                nc.scalar.activation(gbh[:], ps_g[0:128, 0:RB], AF.Copy)
                gbhs.append(gbh)

            # ============ proj1 -> p1e (+ ones col) =========================
            p1e = P.tile([128, NJT, 8, 66], BF16, tag="p1e")
            nc.vector.memset(p1e[:, :, :, 64:65], 1.0)
            for jt in range(NJT):
                ps_p = PS.tile([128, 512], F32, tag="ps")
                nc.tensor.matmul(ps_p[0:128, 0:512],
                                 xTb[:, jt * 128:(jt + 1) * 128], w1Tb[:])
                dst = p1e[:, jt, :, 0:64]
                src = ps_p[0:128, 0:512].rearrange("p (h q) -> p h q", q=64)
                nc.scalar.activation(dst, src, AF.Copy)

            # ============ layer-1 skip (natural layout, + bias) =============
            # skipsb [128 i, 4 ich, 512 hf] bf16
            skipsb = P.tile([128, 4, H1 * F1], BF16, tag="skipsb")
            for ich in range(NIT):
                ps_sk = PS.tile([128, 512], F32, tag="ps")
                nc.tensor.matmul(ps_sk[0:128, 0:512],
                                 xrTb[:, ich * 128:(ich + 1) * 128], ws1Tb[:],
                                 start=True, stop=False)
                nc.tensor.matmul(ps_sk[0:128, 0:512], ones1b[0:1, 0:128],
                                 b1rb[:], start=False, stop=True)
                nc.scalar.activation(skipsb[:, ich, :], ps_sk[0:128, 0:512],
                                     AF.Copy)

            ps_w2 = PS.tile([128, 512], F32, tag="ps")
            for kt in range(4):
                nc.tensor.matmul(ps_w2[0:128, kt * 2:kt * 2 + 2],
                                 w2n[:, kt * 128:(kt + 1) * 128], a2p[:],
                                 start=True, stop=True)
            w2tb = P.tile([128, 8], BF16, tag="w2tb")
            nc.vector.tensor_copy(w2tb[:], ps_w2[0:128, 0:8])

            # ============ layer-1 head loop (i-part numerator) ==============
            hnat = P.tile([128, 4, H1 * F1], BF16, tag="hnat")
            houtb = P.tile([128, 4, RB], BF16, tag="houtb")
            for h in range(H1):
                ps_h = PSN.tile([128, 4, 65], F32, tag="psn")
                for jp in range(NJT // 2):
                    q = WK.tile([128, 2, RB], BF16, tag="q")
                    for l in range(2):
                        jt = 2 * jp + l
                        col = slice(jt * 16 + 8 + h, jt * 16 + 9 + h)
                        nc.vector.tensor_scalar(q[:, l, :], gbhs[h][:],
                                                Wvf[:, col], Dvf[:, col],
                                                ALU.max, ALU.mult)
                    Ft = WK.tile([128, 2, RB], BF16, tag="F")
                    eng = nc.vector if _mode_a(h, jp) else nc.gpsimd
                    eng.tensor_tensor(Ft[:], q[:],
                                      adjTb[:, 2 * jp:2 * jp + 2, :], ALU.mult)
                    for l in range(2):
                        jt = 2 * jp + l
                        mv = p1e[:, jt, h, 0:65]
                        for ich in range(NIT):
                            nc.tensor.matmul(
                                ps_h[:, ich, 0:65],
                                Ft[:, l, ich * 128:(ich + 1) * 128], mv,
                                start=(jt == 0 and ich == 0),
                                stop=(jt == NJT - 1 and ich == NIT - 1))

                # epilogue: h_nat[:, :, h*64:(h+1)*64] = elu(num/den + skip)
                rec = WK.tile([128, 4], F32, tag="rec")
                nc.vector.reciprocal_approx_fast(
                    rec[:], ps_h[:, :, 64:65].rearrange("p a o -> p (a o)"))
                u1 = WK.tile([128, 4, 64], BF16, tag="u1")
                nc.vector.tensor_tensor(
                    u1[:], ps_h[:, :, 0:64],
                    rec[:].unsqueeze(2).to_broadcast((128, 4, 64)),
                    ALU.mult)
                u = WK.tile([128, 4, 64], BF16, tag="u")
                nc.gpsimd.tensor_tensor(
                    u[:], u1[:],
                    skipsb[:].rearrange("p a (g f) -> p a g f", f=64)[:, :, h, :],
                    ALU.add)
                m0 = WK.tile([128, 4, 64], BF16, tag="m0")
                nc.gpsimd.tensor_scalar(m0[:], u[:], 0.0, 1.0, ALU.min, ALU.mult)
                e = WK.tile([128, 4, 64], BF16, tag="e")
                nc.scalar.activation(e[:], m0[:], AF.Exp)
                nc.vector.scalar_tensor_tensor(
                    hnat[:].rearrange("p a (g f) -> p a g f", f=64)[:, :, h, :],
                    e[:], -1.0, u[:], ALU.add, ALU.max)

                # hnat [128 i, 4 ich, hf] -> houtb (h^T) [128 hf, kt, 512 i],
                # transposed per head-pair as soon as both heads are done
                if h % 2 == 1:
                    kt = h // 2
                    ps_t = PST.tile([128, 512], BF16, tag="pst")
                    for ich in range(NIT):
                        nc.tensor.transpose(
                            ps_t[:, ich * 128:(ich + 1) * 128],
                            hnat[:, ich, kt * 128:(kt + 1) * 128], ident[:])
                    nc.scalar.activation(houtb[:, kt, :], ps_t[:], AF.Copy)

            # ============ layer-2 local pieces ==============================
            # S2: s_src2 -> psum row 0, s_tgt2 -> psum row 32
            ps_s2 = PS.tile([128, 512], F32, tag="ps")
            for kt in range(4):
                nc.tensor.matmul(ps_s2[0:1, 0:RB], w2tb[:, kt * 2:kt * 2 + 1],
                                 houtb[:, kt, :], start=(kt == 0), stop=(kt == 3))
            for kt in range(4):
                nc.tensor.matmul(ps_s2[32:33, 0:RB], w2tb[:, kt * 2 + 1:kt * 2 + 2],
                                 houtb[:, kt, :], start=(kt == 0), stop=(kt == 3))
            g2row = P.tile([1, RB], BF16, tag="g2row")
            nc.scalar.activation(g2row[:], ps_s2[0:1, 0:RB], AF.Exp, scale=-0.8)
            stg2 = P.tile([1, RB], F32, tag="stg2")
            nc.scalar.activation(stg2[:], ps_s2[32:33, 0:RB], AF.Copy)

            # proj2^T local [64, 512] in bf16 for the gather
            ps_p2 = PS.tile([128, 512], F32, tag="ps")
            for kt in range(4):
                nc.tensor.matmul(ps_p2[0:64, 0:RB], w2Tb[:, kt, :],
                                 houtb[:, kt, :], start=(kt == 0), stop=(kt == 3))
            p2Tb = P.tile([F1, RB], BF16, tag="p2Tb")
            nc.scalar.activation(p2Tb[:], ps_p2[0:64, 0:RB], AF.Copy)

            # ============ layer-2 epilogue (natural) ========================
            # skip2 natural [128 i, 64] per ich, with b2 folded in via a
            # rank-1 bias matmul
            b2rowb = P.tile([1, F1], BF16, tag="b2rowb")
            nc.vector.tensor_copy(b2rowb[:], b2row[:])
            sk2sb = P.tile([128, 4, F1], F32, tag="sk2sb")
            for ich in range(NIT):
                pssk = PS.tile([128, 512], F32, tag="ps")
                for kt in range(4):
                    nc.tensor.matmul(pssk[0:128, 0:F1],
                                     houtb[:, kt, ich * 128:(ich + 1) * 128],
                                     ws2Tb[:, kt, :],
                                     start=(kt == 0), stop=False)
                nc.tensor.matmul(pssk[0:128, 0:F1], ones1b[0:1, 0:128],
                                 b2rowb[:], start=False, stop=True)
                nc.scalar.activation(sk2sb[:, ich, :], pssk[0:128, 0:F1],
                                     AF.Copy)

            ps_g2 = PS.tile([128, 512], F32, tag="ps")
            nc.tensor.matmul(ps_g2[0:128, 0:RB], ones1b[:], g2row[:])
            g2bc = GB.tile([128, RB], BF16, tag="gb")
            nc.vector.tensor_copy(g2bc[:], ps_g2[0:128, 0:RB])

            # ============ AllGather within batch group ======================
            gin = DR.tile([4, F1 + 1, 128], BF16)
            nc.sync.dma_start(
                gin[:, 0:F1, :].rearrange("s f p -> f s p"),
                p2Tb[:].rearrange("f (s p) -> f s p", p=128))
            stg2b = P.tile([1, RB], BF16, tag="stg2b")
            nc.vector.tensor_copy(stg2b[:], stg2[:])
            nc.sync.dma_start(
                gin[:, F1:F1 + 1, :].rearrange("s o p -> o s p"),
                stg2b[:].rearrange("o (s p) -> o s p", p=128))
            gout = DR.tile([4, 4, F1 + 1, 128], BF16)
            nc.gpsimd.collective_compute(
                "AllGather", ALU.bypass, replica_groups=GROUPS,
                ins=[gin.opt()], outs=[gout.opt()])

            # ============ layer-2 attention =================================
            p2e = P.tile([128, NJT, F1 + 1], BF16, tag="p2e")
            nc.sync.dma_start(
                p2e[:],
                gout.rearrange("c s f p -> p (c s) f"))
            # denominator column must be 1 (on Pool: keeps DVE free for TSPs)
            nc.gpsimd.memset(p2e[:, :, F1:F1 + 1], 1.0)
            st2Tb = P.tile([128, 4, 4], BF16, tag="st2Tb")
            nc.scalar.dma_start(
                st2Tb[:], gout[:, :, F1, :].rearrange("c s p -> p c s"))
            D2v = P.tile([128, 4, 4], F32, tag="D2v")
            nc.scalar.activation(D2v[:], st2Tb[:], AF.Exp, scale=0.2)
            W2vf = P.tile([128, 4, 4], F32, tag="W2vf")
            nc.scalar.activation(W2vf[:], st2Tb[:], AF.Exp, scale=0.8)

            ps_h2 = PSN.tile([128, 4, 65], F32, tag="psn")
            for jp in range(NJT // 2):
                q2 = WK.tile([128, 2, RB], BF16, tag="q")
                for l in range(2):
                    jt = 2 * jp + l
                    c4, s4 = jt // 4, jt % 4
                    nc.vector.tensor_scalar(q2[:, l, :], g2bc[:],
                                            W2vf[:, c4, s4:s4 + 1],
                                            D2v[:, c4, s4:s4 + 1],
                                            ALU.max, ALU.mult)
                F2 = WK.tile([128, 2, RB], BF16, tag="F")
                eng = nc.gpsimd if jp % 8 < 4 else nc.vector
                eng.tensor_tensor(F2[:], q2[:],
                                  adjTb[:, 2 * jp:2 * jp + 2, :], ALU.mult)
                for l in range(2):
                    jt = 2 * jp + l
                    mv = p2e[:, jt, :]
                    for ich in range(NIT):
                        nc.tensor.matmul(
                            ps_h2[:, ich, 0:65],
                            F2[:, l, ich * 128:(ich + 1) * 128], mv,
                            start=(jt == 0 and ich == 0),
                            stop=(jt == NJT - 1 and ich == NIT - 1))

            rec2 = WK.tile([128, 4], F32, tag="rec")
            nc.vector.reciprocal_approx_fast(
                rec2[:], ps_h2[:, :, 64:65].rearrange("p a o -> p (a o)"))
            onat = WK.tile([128, 4, F1], F32, tag="onat")
            t2 = WK.tile([128, 4, F1], F32, tag="t2")
            nc.vector.tensor_tensor(
                t2[:], ps_h2[:, :, 0:64],
                rec2[:].unsqueeze(2).to_broadcast((128, 4, F1)), ALU.mult)
            nc.vector.tensor_tensor(onat[:], t2[:], sk2sb[:], ALU.add)
            nc.sync.dma_start(
                d_out.rearrange("(a p) f -> p a f", p=128), onat[:])

    nc.compile()
    return nc


_NC_CACHE = None


def _get_nc():
    global _NC_CACHE
    if _NC_CACHE is None:
        _NC_CACHE = build_nc()
    return _NC_CACHE


def make_in_maps(x, adj, W1, a_src1, a_tgt1, Wskip1, b1, W2, a_src2, a_tgt2,
                 Wskip2, b2):
    import ml_dtypes
    bf16 = ml_dtypes.bfloat16
    x = np.asarray(x, np.float32)
    adj = np.asarray(adj, np.float32)
    W1 = np.asarray(W1, np.float32)
    W2 = np.asarray(W2, np.float32)
    Wskip1 = np.asarray(Wskip1, np.float32)
    Wskip2 = np.asarray(Wskip2, np.float32)
    in_maps = []
    for c in range(NCORES):
        b, r = c // 4, c % 4
        sl = slice(r * RB, (r + 1) * RB)
        xTf = np.ascontiguousarray(x[b].T)
        xrTf = np.ascontiguousarray(x[b][sl].T)
        in_maps.append({
            "xT": xTf, "xTb": xTf.astype(bf16),
            "xrT": xrTf, "xrTb": xrTf.astype(bf16),
            "adjT": np.ascontiguousarray(adj[b][sl].T).astype(bf16),
            "w1": W1,
            "w1Tb": np.ascontiguousarray(W1.T).astype(bf16),
            "ws1Tb": np.ascontiguousarray(Wskip1.T).astype(bf16),
            "asrc1": np.asarray(a_src1, np.float32),
            "atgt1": np.asarray(a_tgt1, np.float32),
            "b1r": np.asarray(b1, np.float32).reshape(1, -1),
            "w2": W2,
            "w2Tb": np.ascontiguousarray(W2.T).astype(bf16),
            "ws2Tb": np.ascontiguousarray(Wskip2.T).astype(bf16),
            "asrc2": np.asarray(a_src2, np.float32),
            "atgt2": np.asarray(a_tgt2, np.float32),
            "b2": np.asarray(b2, np.float32),
        })
    return in_maps


def kernel(x, adj, W1, a_src1, a_tgt1, Wskip1, b1, W2, a_src2, a_tgt2,
           Wskip2, b2):
    nc = _get_nc()
    in_maps = make_in_maps(x, adj, W1, a_src1, a_tgt1, Wskip1, b1, W2,
                           a_src2, a_tgt2, Wskip2, b2)
    res = run_bass_kernel_spmd(nc, in_maps, core_ids=list(range(NCORES)))
    out = np.empty((BS, N, F1), np.float32)
    for c in range(NCORES):
        b, r = c // 4, c % 4
        out[b, r * RB:(r + 1) * RB, :] = res.results[c]["outN"]
    return out


# revision 48
# speedup vs baseline: 1.3438x; 1.0215x over previous
"""GAT (2-layer graph attention) Trainium2 Bass kernel, 8-core SPMD.

Sharding: data-parallel over batch (2) x row-blocks (4) -> 8 cores.
Core c handles batch b=c//4, output rows R=[512*(c%4), 512*(c%4+1)).

Key algebra: with z = s_src[i]+s_tgt[j], the GAT edge weight
exp(leaky_relu(z, 0.2)) = max(exp(z), exp(0.2 z)).  Softmax rows are
invariant to a per-row scale, so dividing row i by exp(s_src[i]) gives
unnormalized weights F[j,i] = adj[i,j] * D[j] * max(W[j], g[i]) with
  W[j] = exp(0.8 s_tgt[j]),  D[j] = exp(0.2 s_tgt[j]),  g[i] = exp(-0.8 s_src[i])

Layout strategy (v2):
- Host passes adj column-slices pre-transposed and pre-cast to bf16
  (adj is 0/1 so the cast is exact), plus x^T and all weight transposes,
  so no on-chip transposes/casts of inputs are needed.
- The n x n inner work per (head, j-tile) is either one fused
  scalar_tensor_tensor on GPSIMD (max with W, mult by adjT; D folded
  into the stationary operand) or a tensor_scalar + tensor_tensor pair
  on DVE.  g broadcasts ride the idle SP DMA queue.
- The numerator matmul is i-partitioned: stationary = F chunks
  [128j x 128i], moving = proj rows [128j x 65], accumulating into one
  PSUM bank per head ([128, 4, 65]); column 64 gives the softmax
  denominator, which is then a per-partition scalar in the epilogue
  (no reciprocal-broadcast matmuls).

Layer-1 outputs are exchanged within each batch group of 4 cores via a
single AllGather of (proj2^T | s_tgt2) so layer 2 stays row-local.
"""

import os
import sys

for _p in ("/opt/trn_rl_repo", "/root/.axon_site/_ro/trn_rl_repo"):
    if os.path.isdir(_p) and _p not in sys.path:
        sys.path.insert(0, _p)

import numpy as np

import concourse.bass as bass
import concourse.bacc as bacc
import concourse.mybir as mybir
from concourse import tile
from concourse.bass_utils import run_bass_kernel_spmd

F32 = mybir.dt.float32
BF16 = mybir.dt.bfloat16
AF = mybir.ActivationFunctionType
ALU = mybir.AluOpType

BS, N, FIN = 2, 2048, 128
H1, F1 = 8, 64
RB = 512          # row block per core
NJT = N // 128    # 16 j-tiles
NIT = RB // 128   # 4 i-tiles in the row block
NCORES = 8
GROUPS = [[0, 1, 2, 3], [4, 5, 6, 7]]


def _mode_a(h, jp):
    # 'a' pairs: TSP+TT both on DVE; others: TSP on DVE, TT on Pool
    return (h * 8 + jp) % 10 < 3


def build_nc():
    nc = bacc.Bacc("TRN2", target_bir_lowering=False, debug=False,
                   num_devices=NCORES)

    # ---- per-core DRAM I/O (host pre-transposes / pre-casts) ----
    d_xT = nc.declare_dram_parameter("xT", [FIN, N], F32, isOutput=False)
    d_xTb = nc.declare_dram_parameter("xTb", [FIN, N], BF16, isOutput=False)
    d_xrT = nc.declare_dram_parameter("xrT", [FIN, RB], F32, isOutput=False)
    d_xrTb = nc.declare_dram_parameter("xrTb", [FIN, RB], BF16, isOutput=False)
    d_adjT = nc.declare_dram_parameter("adjT", [N, RB], BF16, isOutput=False)
    d_w1n = nc.declare_dram_parameter("w1", [H1 * F1, FIN], F32, isOutput=False)
    d_w1Tb = nc.declare_dram_parameter("w1Tb", [FIN, H1 * F1], BF16,
                                       isOutput=False)
    d_ws1Tb = nc.declare_dram_parameter("ws1Tb", [FIN, H1 * F1], BF16,
                                        isOutput=False)
    d_as1 = nc.declare_dram_parameter("asrc1", [H1, F1], F32, isOutput=False)
    d_at1 = nc.declare_dram_parameter("atgt1", [H1, F1], F32, isOutput=False)
    d_b1r = nc.declare_dram_parameter("b1r", [1, H1 * F1], BF16, isOutput=False)
    d_w2 = nc.declare_dram_parameter("w2", [F1, H1 * F1], F32, isOutput=False)
    d_w2Tb = nc.declare_dram_parameter("w2Tb", [H1 * F1, F1], BF16,
                                       isOutput=False)
    d_ws2Tb = nc.declare_dram_parameter("ws2Tb", [H1 * F1, F1], BF16,
                                        isOutput=False)
    d_as2 = nc.declare_dram_parameter("asrc2", [1, F1], F32, isOutput=False)
    d_at2 = nc.declare_dram_parameter("atgt2", [1, F1], F32, isOutput=False)
    d_b2 = nc.declare_dram_parameter("b2", [F1], F32, isOutput=False)
    # output: natural row-block [512, 64]
    d_out = nc.declare_dram_parameter("outN", [RB, F1], F32, isOutput=True)

    with tile.TileContext(nc) as tc:
        with (
            tc.tile_pool(name="persist", bufs=1) as P,
            tc.tile_pool(name="work", bufs=8) as WK,
            tc.tile_pool(name="gbp", bufs=3) as GB,
            tc.tile_pool(name="ps", bufs=3, space="PSUM") as PS,
            tc.tile_pool(name="psnum", bufs=4, space="PSUM") as PSN,
            tc.tile_pool(name="pst", bufs=1, space="PSUM") as PST,
            tc.tile_pool(name="dram", bufs=1, space="DRAM") as DR,
        ):
            # ============ loads (all natural-layout now) ====================
            # SP queue: xT first (scores path), then xTb, then adjT half
            xT = P.tile([128, N], F32, tag="xT")
            xTb = P.tile([128, N], BF16, tag="xTb")
            nc.sync.dma_start(xT[:, 0:N // 2], d_xT[:, 0:N // 2])
            nc.sync.dma_start(xTb[:, 0:N // 2], d_xTb[:, 0:N // 2])
            nc.sync.dma_start(xT[:, N // 2:N], d_xT[:, N // 2:N])
            nc.sync.dma_start(xTb[:, N // 2:N], d_xTb[:, N // 2:N])
            # Act queue: adjT other half + xr + small weights
            adjTb = P.tile([128, NJT, RB], BF16, tag="adjTb")
            nc.sync.dma_start(
                adjTb[:, 0:NJT // 2, :],
                d_adjT[0:N // 2, :].rearrange("(t p) i -> p t i", p=128))
            nc.sync.dma_start(
                adjTb[:, NJT // 2:NJT, :],
                d_adjT[N // 2:N, :].rearrange("(t p) i -> p t i", p=128))
            xrT = P.tile([128, RB], F32, tag="xrT")
            nc.scalar.dma_start(xrT[:], d_xrT[:, :])
            xrTb = P.tile([128, RB], BF16, tag="xrTb")
            nc.scalar.dma_start(xrTb[:], d_xrTb[:, :])
            # selector tiles sel_h [16, 128] (row h ones) for g broadcasts,
            # emitted before the Pool DMA chain so they clear the queue early
            ones16 = P.tile([16, 128], BF16, tag="ones16")
            nc.vector.memset(ones16[:], 1.0)
            sel = P.tile([16, H1 * 128], BF16, tag="sel")
            for h in range(H1):
                nc.gpsimd.affine_select(sel[:, h * 128:(h + 1) * 128],
                                        ones16[:], [[0, 128]], ALU.is_equal,
                                        0.0, base=-h, channel_multiplier=1)
            # Pool queue: proj1 weights first, then a-vectors
            w1Tb = P.tile([128, H1 * F1], BF16, tag="w1Tb")
            nc.gpsimd.dma_start(w1Tb[:], d_w1Tb[:, :])
            w1n = P.tile([128, 4, FIN], F32, tag="w1n")
            nc.gpsimd.dma_start(w1n[:], d_w1n.rearrange("(k p) c -> p k c", p=128))
            a1sT = P.tile([128, H1], F32, tag="a1sT")
            nc.gpsimd.dma_start(a1sT[0:F1, :], d_as1.rearrange("h f -> f h"))
            nc.gpsimd.dma_start(a1sT[F1:2 * F1, :], d_as1.rearrange("h f -> f h"))
            a1tT = P.tile([128, H1], F32, tag="a1tT")
            nc.gpsimd.dma_start(a1tT[0:F1, :], d_at1.rearrange("h f -> f h"))
            nc.gpsimd.dma_start(a1tT[F1:2 * F1, :], d_at1.rearrange("h f -> f h"))
            ws1Tb = P.tile([128, H1 * F1], BF16, tag="ws1Tb")
            nc.gpsimd.dma_start(ws1Tb[:], d_ws1Tb[:, :])
            b1rb = P.tile([1, H1 * F1], BF16, tag="b1rb")
            nc.sync.dma_start(b1rb[:], d_b1r[:, :])
            # L2 weights ride the SP queue later (needed only at L2 time)
            w2n = P.tile([F1, H1 * F1], F32, tag="w2n")
            nc.sync.dma_start(w2n[:], d_w2[:, :])
            w2Tb = P.tile([128, 4, F1], BF16, tag="w2Tb")
            nc.sync.dma_start(w2Tb[:], d_w2Tb.rearrange("(k p) f -> p k f", p=128))
            ws2Tb = P.tile([128, 4, F1], BF16, tag="ws2Tb")
            nc.sync.dma_start(ws2Tb[:], d_ws2Tb.rearrange("(k p) f -> p k f", p=128))
            a2p = P.tile([F1, 2], F32, tag="a2p")
            nc.sync.dma_start(a2p[:, 0:1], d_as2.rearrange("o f -> f o"))
            nc.sync.dma_start(a2p[:, 1:2], d_at2.rearrange("o f -> f o"))
            b2row = P.tile([1, F1], F32, tag="b2row")
            nc.sync.dma_start(b2row[:], d_b2.ap().rearrange("(o f) -> o f", o=1))

            ones1b = P.tile([1, 128], BF16, tag="ones1b")
            nc.vector.memset(ones1b[:], 1.0)
            # identity (bf16) for PE transposes of h
            onesq = P.tile([128, 128], BF16, tag="onesq")
            nc.vector.memset(onesq[:], 1.0)
            ident = P.tile([128, 128], BF16, tag="ident")
            nc.gpsimd.affine_select(ident[:], onesq[:], [[-1, 128]],
                                    ALU.is_equal, 0.0, base=0,
                                    channel_multiplier=1)

            # ============ scores (exact fp32) ===============================
            # w1tilde [c=128, 16]: col h = W1_h^T a_src1[h], col 8+h tgt
            ps_wt = PS.tile([128, 512], F32, tag="ps")
            for h in range(H1):
                kt, pr = (h * F1) // 128, (h * F1) % 128
                w1slc = w1n[pr:pr + F1, kt, :]
                nc.tensor.matmul(ps_wt[0:128, h:h + 1], w1slc,
                                 a1sT[pr:pr + F1, h:h + 1])
                nc.tensor.matmul(ps_wt[0:128, 8 + h:9 + h], w1slc,
                                 a1tT[pr:pr + F1, h:h + 1])
            w1t = P.tile([128, 16], F32, tag="w1t")
            nc.vector.tensor_copy(w1t[:], ps_wt[0:128, 0:16])

            # S1T [j(128 x 16 chunks), 16] = x @ w1tilde
            ps_s1t = PS.tile([128, 512], F32, tag="ps")
            for jc in range(NJT):
                nc.tensor.matmul(ps_s1t[0:128, jc * 16:(jc + 1) * 16],
                                 xT[:, jc * 128:(jc + 1) * 128], w1t[:])
            s1T = P.tile([128, NJT * 16], F32, tag="s1T")
            Wvf = P.tile([128, NJT * 16], F32, tag="Wvf")
            Dvf = P.tile([128, NJT * 16], F32, tag="Dvf")
            HC = NJT * 8
            for hh in range(2):
                cs = slice(hh * HC, (hh + 1) * HC)
                nc.vector.tensor_copy(s1T[:, cs], ps_s1t[0:128, cs])
                nc.scalar.activation(Wvf[:, cs], s1T[:, cs], AF.Exp, scale=0.8)
                nc.scalar.activation(Dvf[:, cs], s1T[:, cs], AF.Exp, scale=0.2)

            # s_src rows for our block -> g (bf16) [16, 512]
            ps_s1r = PS.tile([128, 512], F32, tag="ps")
            nc.tensor.matmul(ps_s1r[0:16, 0:RB], w1t[:], xrT[:])
            g1b = P.tile([16, RB], BF16, tag="g1b")
            nc.scalar.activation(g1b[:], ps_s1r[0:16, 0:RB], AF.Exp, scale=-0.8)



            # g broadcasts: PE selector matmul + copy out of PSUM
            gbhs = []
            for h in range(H1):
                ps_g = PS.tile([128, 512], F32, tag="ps")
                nc.tensor.matmul(ps_g[0:128, 0:RB],
                                 sel[:, h * 128:(h + 1) * 128], g1b[:])
                gbh = P.tile([128, RB], BF16, tag=f"gbh{h}", name=f"gbh{h}")
                nc.scalar.activation(gbh[:], ps_g[0:128, 0:RB], AF.Copy)
                gbhs.append(gbh)

            # ============ proj1 -> p1e (+ ones col) =========================
            p1e = P.tile([128, NJT, 8, 66], BF16, tag="p1e")
            nc.vector.memset(p1e[:, :, :, 64:65], 1.0)
            for jt in range(NJT):
                ps_p = PS.tile([128, 512], F32, tag="ps")
                nc.tensor.matmul(ps_p[0:128, 0:512],
                                 xTb[:, jt * 128:(jt + 1) * 128], w1Tb[:])
                dst = p1e[:, jt, :, 0:64]
                src = ps_p[0:128, 0:512].rearrange("p (h q) -> p h q", q=64)
                nc.scalar.activation(dst, src, AF.Copy)

            # ============ layer-1 skip (natural layout, + bias) =============
            # skipsb [128 i, 4 ich, 512 hf] bf16
            skipsb = P.tile([128, 4, H1 * F1], BF16, tag="skipsb")
            for ich in range(NIT):
                ps_sk = PS.tile([128, 512], F32, tag="ps")
                nc.tensor.matmul(ps_sk[0:128, 0:512],
                                 xrTb[:, ich * 128:(ich + 1) * 128], ws1Tb[:],
                                 start=True, stop=False)
                nc.tensor.matmul(ps_sk[0:128, 0:512], ones1b[0:1, 0:128],
                                 b1rb[:], start=False, stop=True)
                nc.scalar.activation(skipsb[:, ich, :], ps_sk[0:128, 0:512],
                                     AF.Copy)

            ps_w2 = PS.tile([128, 512], F32, tag="ps")
            for kt in range(4):
                nc.tensor.matmul(ps_w2[0:128, kt * 2:kt * 2 + 2],
                                 w2n[:, kt * 128:(kt + 1) * 128], a2p[:],
                                 start=True, stop=True)
            w2tb = P.tile([128, 8], BF16, tag="w2tb")
            nc.vector.tensor_copy(w2tb[:], ps_w2[0:128, 0:8])

            # ============ layer-1 head loop (i-part numerator) ==============
            hnat = P.tile([128, 4, H1 * F1], BF16, tag="hnat")
            houtb = P.tile([128, 4, RB], BF16, tag="houtb")
            for h in range(H1):
                ps_h = PSN.tile([128, 4, 65], F32, tag="psn")
                for jp in range(NJT // 2):
                    q = WK.tile([128, 2, RB], BF16, tag="q")
                    for l in range(2):
                        jt = 2 * jp + l
                        col = slice(jt * 16 + 8 + h, jt * 16 + 9 + h)
                        nc.vector.tensor_scalar(q[:, l, :], gbhs[h][:],
                                                Wvf[:, col], Dvf[:, col],
                                                ALU.max, ALU.mult)
                    Ft = WK.tile([128, 2, RB], BF16, tag="F")
                    eng = nc.vector if _mode_a(h, jp) else nc.gpsimd
                    eng.tensor_tensor(Ft[:], q[:],
                                      adjTb[:, 2 * jp:2 * jp + 2, :], ALU.mult)
                    for l in range(2):
                        jt = 2 * jp + l
                        mv = p1e[:, jt, h, 0:65]
                        for ich in range(NIT):
                            nc.tensor.matmul(
                                ps_h[:, ich, 0:65],
                                Ft[:, l, ich * 128:(ich + 1) * 128], mv,
                                start=(jt == 0 and ich == 0),
                                stop=(jt == NJT - 1 and ich == NIT - 1))

                # epilogue: h_nat[:, :, h*64:(h+1)*64] = elu(num/den + skip)
                rec = WK.tile([128, 4], F32, tag="rec")
                nc.vector.reciprocal_approx_fast(
                    rec[:], ps_h[:, :, 64:65].rearrange("p a o -> p (a o)"))
                u1 = WK.tile([128, 4, 64], BF16, tag="u1")
                nc.vector.tensor_tensor(
                    u1[:], ps_h[:, :, 0:64],
                    rec[:].unsqueeze(2).to_broadcast((128, 4, 64)),
                    ALU.mult)
                u = WK.tile([128, 4, 64], BF16, tag="u")
                nc.gpsimd.tensor_tensor(
                    u[:], u1[:],
                    skipsb[:].rearrange("p a (g f) -> p a g f", f=64)[:, :, h, :],
                    ALU.add)
                m0 = WK.tile([128, 4, 64], BF16, tag="m0")
                nc.gpsimd.tensor_scalar(m0[:], u[:], 0.0, 1.0, ALU.min, ALU.mult)
                e = WK.tile([128, 4, 64], BF16, tag="e")
                nc.scalar.activation(e[:], m0[:], AF.Exp)
                nc.vector.scalar_tensor_tensor(
                    hnat[:].rearrange("p a (g f) -> p a g f", f=64)[:, :, h, :],
                    e[:], -1.0, u[:], ALU.add, ALU.max)

                # hnat [128 i, 4 ich, hf] -> houtb (h^T) [128 hf, kt, 512 i],
                # transposed per head-pair as soon as both heads are done
                if h % 2 == 1:
                    kt = h // 2
                    ps_t = PST.tile([128, 512], BF16, tag="pst")
                    for ich in range(NIT):
                        nc.tensor.transpose(
                            ps_t[:, ich * 128:(ich + 1) * 128],
                            hnat[:, ich, kt * 128:(kt + 1) * 128], ident[:])
                    if kt == 3:
                        nc.vector.tensor_copy(houtb[:, kt, :], ps_t[:])
                    else:
                        nc.scalar.activation(houtb[:, kt, :], ps_t[:], AF.Copy)

            # ============ layer-2 local pieces ==============================
            # S2: s_src2 -> psum row 0, s_tgt2 -> psum row 32
            ps_s2 = PS.tile([128, 512], F32, tag="ps")
            for kt in range(4):
                nc.tensor.matmul(ps_s2[0:1, 0:RB], w2tb[:, kt * 2:kt * 2 + 1],
                                 houtb[:, kt, :], start=(kt == 0), stop=(kt == 3))
            for kt in range(4):
                nc.tensor.matmul(ps_s2[32:33, 0:RB], w2tb[:, kt * 2 + 1:kt * 2 + 2],
                                 houtb[:, kt, :], start=(kt == 0), stop=(kt == 3))
            g2row = P.tile([1, RB], BF16, tag="g2row")
            nc.scalar.activation(g2row[:], ps_s2[0:1, 0:RB], AF.Exp, scale=-0.8)
            stg2 = P.tile([1, RB], F32, tag="stg2")
            nc.scalar.activation(stg2[:], ps_s2[32:33, 0:RB], AF.Copy)

            # proj2^T local [64, 512] in bf16 for the gather
            ps_p2 = PS.tile([128, 512], F32, tag="ps")
            for kt in range(4):
                nc.tensor.matmul(ps_p2[0:64, 0:RB], w2Tb[:, kt, :],
                                 houtb[:, kt, :], start=(kt == 0), stop=(kt == 3))
            p2Tb = P.tile([F1, RB], BF16, tag="p2Tb")
            nc.vector.tensor_copy(p2Tb[:], ps_p2[0:64, 0:RB])

            # ============ layer-2 epilogue (natural) ========================
            # skip2 natural [128 i, 64] per ich, with b2 folded in via a
            # rank-1 bias matmul
            b2rowb = P.tile([1, F1], BF16, tag="b2rowb")
            nc.vector.tensor_copy(b2rowb[:], b2row[:])
            sk2sb = P.tile([128, 4, F1], F32, tag="sk2sb")
            for ich in range(NIT):
                pssk = PS.tile([128, 512], F32, tag="ps")
                for kt in range(4):
                    nc.tensor.matmul(pssk[0:128, 0:F1],
                                     houtb[:, kt, ich * 128:(ich + 1) * 128],
                                     ws2Tb[:, kt, :],
                                     start=(kt == 0), stop=False)
                nc.tensor.matmul(pssk[0:128, 0:F1], ones1b[0:1, 0:128],
                                 b2rowb[:], start=False, stop=True)
                nc.vector.tensor_copy(sk2sb[:, ich, :], pssk[0:128, 0:F1])

            ps_g2 = PS.tile([128, 512], F32, tag="ps")
            nc.tensor.matmul(ps_g2[0:128, 0:RB], ones1b[:], g2row[:])
            g2bc = GB.tile([128, RB], BF16, tag="gb")
            nc.vector.tensor_copy(g2bc[:], ps_g2[0:128, 0:RB])

            # ============ AllGather within batch group ======================
            # payload columns: [proj2 (64) | ones (1) | s_tgt2 (1)] so the
            # receiver's moving operand [proj|1] is a contiguous 65-slice
            # and no post-gather memset is needed
            gin = DR.tile([4, F1 + 2, 128], BF16)
            nc.sync.dma_start(
                gin[:, 0:F1, :].rearrange("s f p -> f s p"),
                p2Tb[:].rearrange("f (s p) -> f s p", p=128))
            ones512b = P.tile([1, RB], BF16, tag="ones512b")
            nc.vector.memset(ones512b[:], 1.0)
            nc.sync.dma_start(
                gin[:, F1:F1 + 1, :].rearrange("s o p -> o s p"),
                ones512b[:].rearrange("o (s p) -> o s p", p=128))
            stg2b = P.tile([1, RB], BF16, tag="stg2b")
            nc.vector.tensor_copy(stg2b[:], stg2[:])
            nc.sync.dma_start(
                gin[:, F1 + 1:F1 + 2, :].rearrange("s o p -> o s p"),
                stg2b[:].rearrange("o (s p) -> o s p", p=128))
            gout = DR.tile([4, 4, F1 + 2, 128], BF16)
            nc.gpsimd.collective_compute(
                "AllGather", ALU.bypass, replica_groups=GROUPS,
                ins=[gin.opt()], outs=[gout.opt()])

            # ============ layer-2 attention =================================
            p2e = P.tile([128, NJT, F1 + 2], BF16, tag="p2e")
            nc.sync.dma_start(
                p2e[:, 0:NJT // 2, :],
                gout[0:2].rearrange("c s f p -> p (c s) f"))
            nc.sync.dma_start(
                p2e[:, NJT // 2:NJT, :],
                gout[2:4].rearrange("c s f p -> p (c s) f"))
            st2Tb = P.tile([128, 4, 4], BF16, tag="st2Tb")
            nc.scalar.dma_start(
                st2Tb[:], gout[:, :, F1 + 1, :].rearrange("c s p -> p c s"))
            D2v = P.tile([128, 4, 4], F32, tag="D2v")
            nc.scalar.activation(D2v[:], st2Tb[:], AF.Exp, scale=0.2)
            W2vf = P.tile([128, 4, 4], F32, tag="W2vf")
            nc.scalar.activation(W2vf[:], st2Tb[:], AF.Exp, scale=0.8)

            ps_h2 = PSN.tile([128, 4, 65], F32, tag="psn")
            for jp in range(NJT // 2):
                q2 = WK.tile([128, 2, RB], BF16, tag="q")
                for l in range(2):
                    jt = 2 * jp + l
                    c4, s4 = jt // 4, jt % 4
                    nc.vector.tensor_scalar(q2[:, l, :], g2bc[:],
                                            W2vf[:, c4, s4:s4 + 1],
                                            D2v[:, c4, s4:s4 + 1],
                                            ALU.max, ALU.mult)
                F2 = WK.tile([128, 2, RB], BF16, tag="F")
                eng = nc.gpsimd if jp % 8 < 4 else nc.vector
                eng.tensor_tensor(F2[:], q2[:],
                                  adjTb[:, 2 * jp:2 * jp + 2, :], ALU.mult)
                for l in range(2):
                    jt = 2 * jp + l
                    mv = p2e[:, jt, 0:F1 + 1]
                    for ich in range(NIT):
                        nc.tensor.matmul(
                            ps_h2[:, ich, 0:65],
                            F2[:, l, ich * 128:(ich + 1) * 128], mv,
                            start=(jt == 0 and ich == 0),
                            stop=(jt == NJT - 1 and ich == NIT - 1))

            rec2 = WK.tile([128, 4], F32, tag="rec")
            nc.vector.reciprocal_approx_fast(
                rec2[:], ps_h2[:, :, 64:65].rearrange("p a o -> p (a o)"))
            onat = WK.tile([128, 4, F1], F32, tag="onat")
            t2 = WK.tile([128, 4, F1], F32, tag="t2")
            nc.vector.tensor_tensor(
                t2[:], ps_h2[:, :, 0:64],
                rec2[:].unsqueeze(2).to_broadcast((128, 4, F1)), ALU.mult)
            nc.vector.tensor_tensor(onat[:], t2[:], sk2sb[:], ALU.add)
            nc.sync.dma_start(
                d_out.rearrange("(a p) f -> p a f", p=128), onat[:])

    nc.compile()
    return nc


_NC_CACHE = None


def _get_nc():
    global _NC_CACHE
    if _NC_CACHE is None:
        _NC_CACHE = build_nc()
    return _NC_CACHE


def make_in_maps(x, adj, W1, a_src1, a_tgt1, Wskip1, b1, W2, a_src2, a_tgt2,
                 Wskip2, b2):
    import ml_dtypes
    bf16 = ml_dtypes.bfloat16
    x = np.asarray(x, np.float32)
    adj = np.asarray(adj, np.float32)
    W1 = np.asarray(W1, np.float32)
    W2 = np.asarray(W2, np.float32)
    Wskip1 = np.asarray(Wskip1, np.float32)
    Wskip2 = np.asarray(Wskip2, np.float32)
    in_maps = []
    for c in range(NCORES):
        b, r = c // 4, c % 4
        sl = slice(r * RB, (r + 1) * RB)
        xTf = np.ascontiguousarray(x[b].T)
        xrTf = np.ascontiguousarray(x[b][sl].T)
        in_maps.append({
            "xT": xTf, "xTb": xTf.astype(bf16),
            "xrT": xrTf, "xrTb": xrTf.astype(bf16),
            "adjT": np.ascontiguousarray(adj[b][sl].T).astype(bf16),
            "w1": W1,
            "w1Tb": np.ascontiguousarray(W1.T).astype(bf16),
            "ws1Tb": np.ascontiguousarray(Wskip1.T).astype(bf16),
            "asrc1": np.asarray(a_src1, np.float32),
            "atgt1": np.asarray(a_tgt1, np.float32),
            "b1r": np.asarray(b1, np.float32).reshape(1, -1).astype(bf16),
            "w2": W2,
            "w2Tb": np.ascontiguousarray(W2.T).astype(bf16),
            "ws2Tb": np.ascontiguousarray(Wskip2.T).astype(bf16),
            "asrc2": np.asarray(a_src2, np.float32),
            "atgt2": np.asarray(a_tgt2, np.float32),
            "b2": np.asarray(b2, np.float32),
        })
    return in_maps


def kernel(x, adj, W1, a_src1, a_tgt1, Wskip1, b1, W2, a_src2, a_tgt2,
           Wskip2, b2):
    nc = _get_nc()
    in_maps = make_in_maps(x, adj, W1, a_src1, a_tgt1, Wskip1, b1, W2,
                           a_src2, a_tgt2, Wskip2, b2)
    res = run_bass_kernel_spmd(nc, in_maps, core_ids=list(range(NCORES)))
    out = np.empty((BS, N, F1), np.float32)
    for c in range(NCORES):
        b, r = c // 4, c % 4
        out[b, r * RB:(r + 1) * RB, :] = res.results[c]["outN"]
    return out


# revision 49
# speedup vs baseline: 1.3439x; 1.0001x over previous
"""GAT (2-layer graph attention) Trainium2 Bass kernel, 8-core SPMD.

Sharding: data-parallel over batch (2) x row-blocks (4) -> 8 cores.
Core c handles batch b=c//4, output rows R=[512*(c%4), 512*(c%4+1)).

Key algebra: with z = s_src[i]+s_tgt[j], the GAT edge weight
exp(leaky_relu(z, 0.2)) = max(exp(z), exp(0.2 z)).  Softmax rows are
invariant to a per-row scale, so dividing row i by exp(s_src[i]) gives
unnormalized weights F[j,i] = adj[i,j] * D[j] * max(W[j], g[i]) with
  W[j] = exp(0.8 s_tgt[j]),  D[j] = exp(0.2 s_tgt[j]),  g[i] = exp(-0.8 s_src[i])

Layout strategy (v2):
- Host passes adj column-slices pre-transposed and pre-cast to bf16
  (adj is 0/1 so the cast is exact), plus x^T and all weight transposes,
  so no on-chip transposes/casts of inputs are needed.
- The n x n inner work per (head, j-tile) is either one fused
  scalar_tensor_tensor on GPSIMD (max with W, mult by adjT; D folded
  into the stationary operand) or a tensor_scalar + tensor_tensor pair
  on DVE.  g broadcasts ride the idle SP DMA queue.
- The numerator matmul is i-partitioned: stationary = F chunks
  [128j x 128i], moving = proj rows [128j x 65], accumulating into one
  PSUM bank per head ([128, 4, 65]); column 64 gives the softmax
  denominator, which is then a per-partition scalar in the epilogue
  (no reciprocal-broadcast matmuls).

Layer-1 outputs are exchanged within each batch group of 4 cores via a
single AllGather of (proj2^T | s_tgt2) so layer 2 stays row-local.
"""

import os
import sys

for _p in ("/opt/trn_rl_repo", "/root/.axon_site/_ro/trn_rl_repo"):
    if os.path.isdir(_p) and _p not in sys.path:
        sys.path.insert(0, _p)

import numpy as np

import concourse.bass as bass
import concourse.bacc as bacc
import concourse.mybir as mybir
from concourse import tile
from concourse.bass_utils import run_bass_kernel_spmd

F32 = mybir.dt.float32
BF16 = mybir.dt.bfloat16
AF = mybir.ActivationFunctionType
ALU = mybir.AluOpType

BS, N, FIN = 2, 2048, 128
H1, F1 = 8, 64
RB = 512          # row block per core
NJT = N // 128    # 16 j-tiles
NIT = RB // 128   # 4 i-tiles in the row block
NCORES = 8
GROUPS = [[0, 1, 2, 3], [4, 5, 6, 7]]


def _mode_a(h, jp):
    # 'a' pairs: TSP+TT both on DVE; others: TSP on DVE, TT on Pool
    return (h * 8 + jp) % 10 < 3


def build_nc():
    nc = bacc.Bacc("TRN2", target_bir_lowering=False, debug=False,
                   num_devices=NCORES)

    # ---- per-core DRAM I/O (host pre-transposes / pre-casts) ----
    d_xT = nc.declare_dram_parameter("xT", [FIN, N], F32, isOutput=False)
    d_xTb = nc.declare_dram_parameter("xTb", [FIN, N], BF16, isOutput=False)
    d_xrT = nc.declare_dram_parameter("xrT", [FIN, RB], F32, isOutput=False)
    d_xrTb = nc.declare_dram_parameter("xrTb", [FIN, RB], BF16, isOutput=False)
    d_adjT = nc.declare_dram_parameter("adjT", [N, RB], BF16, isOutput=False)
    d_w1n = nc.declare_dram_parameter("w1", [H1 * F1, FIN], F32, isOutput=False)
    d_w1Tb = nc.declare_dram_parameter("w1Tb", [FIN, H1 * F1], BF16,
                                       isOutput=False)
    d_ws1Tb = nc.declare_dram_parameter("ws1Tb", [FIN, H1 * F1], BF16,
                                        isOutput=False)
    d_as1 = nc.declare_dram_parameter("asrc1", [H1, F1], F32, isOutput=False)
    d_at1 = nc.declare_dram_parameter("atgt1", [H1, F1], F32, isOutput=False)
    d_b1r = nc.declare_dram_parameter("b1r", [1, H1 * F1], BF16, isOutput=False)
    d_w2 = nc.declare_dram_parameter("w2", [F1, H1 * F1], F32, isOutput=False)
    d_w2Tb = nc.declare_dram_parameter("w2Tb", [H1 * F1, F1], BF16,
                                       isOutput=False)
    d_ws2Tb = nc.declare_dram_parameter("ws2Tb", [H1 * F1, F1], BF16,
                                        isOutput=False)
    d_as2 = nc.declare_dram_parameter("asrc2", [1, F1], F32, isOutput=False)
    d_at2 = nc.declare_dram_parameter("atgt2", [1, F1], F32, isOutput=False)
    d_b2 = nc.declare_dram_parameter("b2", [F1], F32, isOutput=False)
    # output: natural row-block [512, 64]
    d_out = nc.declare_dram_parameter("outN", [RB, F1], F32, isOutput=True)

    with tile.TileContext(nc) as tc:
        with (
            tc.tile_pool(name="persist", bufs=1) as P,
            tc.tile_pool(name="work", bufs=8) as WK,
            tc.tile_pool(name="gbp", bufs=3) as GB,
            tc.tile_pool(name="ps", bufs=3, space="PSUM") as PS,
            tc.tile_pool(name="psnum", bufs=4, space="PSUM") as PSN,
            tc.tile_pool(name="pst", bufs=1, space="PSUM") as PST,
            tc.tile_pool(name="dram", bufs=1, space="DRAM") as DR,
        ):
            # ============ loads (all natural-layout now) ====================
            # SP queue: xT first (scores path), then xTb, then adjT half
            xT = P.tile([128, N], F32, tag="xT")
            xTb = P.tile([128, N], BF16, tag="xTb")
            nc.sync.dma_start(xT[:, 0:N // 2], d_xT[:, 0:N // 2])
            nc.sync.dma_start(xTb[:, 0:N // 2], d_xTb[:, 0:N // 2])
            nc.sync.dma_start(xT[:, N // 2:N], d_xT[:, N // 2:N])
            nc.sync.dma_start(xTb[:, N // 2:N], d_xTb[:, N // 2:N])
            # Act queue: adjT other half + xr + small weights
            adjTb = P.tile([128, NJT, RB], BF16, tag="adjTb")
            nc.sync.dma_start(
                adjTb[:, 0:NJT // 2, :],
                d_adjT[0:N // 2, :].rearrange("(t p) i -> p t i", p=128))
            nc.sync.dma_start(
                adjTb[:, NJT // 2:NJT, :],
                d_adjT[N // 2:N, :].rearrange("(t p) i -> p t i", p=128))
            xrT = P.tile([128, RB], F32, tag="xrT")
            nc.scalar.dma_start(xrT[:], d_xrT[:, :])
            xrTb = P.tile([128, RB], BF16, tag="xrTb")
            nc.scalar.dma_start(xrTb[:], d_xrTb[:, :])
            # selector tiles sel_h [16, 128] (row h ones) for g broadcasts,
            # emitted before the Pool DMA chain so they clear the queue early
            ones16 = P.tile([16, 128], BF16, tag="ones16")
            nc.vector.memset(ones16[:], 1.0)
            sel = P.tile([16, H1 * 128], BF16, tag="sel")
            for h in range(H1):
                nc.gpsimd.affine_select(sel[:, h * 128:(h + 1) * 128],
                                        ones16[:], [[0, 128]], ALU.is_equal,
                                        0.0, base=-h, channel_multiplier=1)
            # Pool queue: proj1 weights first, then a-vectors
            w1Tb = P.tile([128, H1 * F1], BF16, tag="w1Tb")
            nc.gpsimd.dma_start(w1Tb[:], d_w1Tb[:, :])
            w1n = P.tile([128, 4, FIN], F32, tag="w1n")
            nc.gpsimd.dma_start(w1n[:], d_w1n.rearrange("(k p) c -> p k c", p=128))
            a1sT = P.tile([128, H1], F32, tag="a1sT")
            nc.gpsimd.dma_start(a1sT[0:F1, :], d_as1.rearrange("h f -> f h"))
            nc.gpsimd.dma_start(a1sT[F1:2 * F1, :], d_as1.rearrange("h f -> f h"))
            a1tT = P.tile([128, H1], F32, tag="a1tT")
            nc.gpsimd.dma_start(a1tT[0:F1, :], d_at1.rearrange("h f -> f h"))
            nc.gpsimd.dma_start(a1tT[F1:2 * F1, :], d_at1.rearrange("h f -> f h"))
            ws1Tb = P.tile([128, H1 * F1], BF16, tag="ws1Tb")
            nc.gpsimd.dma_start(ws1Tb[:], d_ws1Tb[:, :])
            b1rb = P.tile([1, H1 * F1], BF16, tag="b1rb")
            nc.sync.dma_start(b1rb[:], d_b1r[:, :])
            # L2 weights ride the SP queue later (needed only at L2 time)
            w2n = P.tile([F1, H1 * F1], F32, tag="w2n")
            nc.sync.dma_start(w2n[:], d_w2[:, :])
            w2Tb = P.tile([128, 4, F1], BF16, tag="w2Tb")
            nc.sync.dma_start(w2Tb[:], d_w2Tb.rearrange("(k p) f -> p k f", p=128))
            ws2Tb = P.tile([128, 4, F1], BF16, tag="ws2Tb")
            nc.sync.dma_start(ws2Tb[:], d_ws2Tb.rearrange("(k p) f -> p k f", p=128))
            a2p = P.tile([F1, 2], F32, tag="a2p")
            nc.sync.dma_start(a2p[:, 0:1], d_as2.rearrange("o f -> f o"))
            nc.sync.dma_start(a2p[:, 1:2], d_at2.rearrange("o f -> f o"))
            b2row = P.tile([1, F1], F32, tag="b2row")
            nc.sync.dma_start(b2row[:], d_b2.ap().rearrange("(o f) -> o f", o=1))

            ones1b = P.tile([1, 128], BF16, tag="ones1b")
            nc.vector.memset(ones1b[:], 1.0)
            # identity (bf16) for PE transposes of h
            onesq = P.tile([128, 128], BF16, tag="onesq")
            nc.vector.memset(onesq[:], 1.0)
            ident = P.tile([128, 128], BF16, tag="ident")
            nc.gpsimd.affine_select(ident[:], onesq[:], [[-1, 128]],
                                    ALU.is_equal, 0.0, base=0,
                                    channel_multiplier=1)

            # ============ scores (exact fp32) ===============================
            # w1tilde [c=128, 16]: col h = W1_h^T a_src1[h], col 8+h tgt
            ps_wt = PS.tile([128, 512], F32, tag="ps")
            for h in range(H1):
                kt, pr = (h * F1) // 128, (h * F1) % 128
                w1slc = w1n[pr:pr + F1, kt, :]
                nc.tensor.matmul(ps_wt[0:128, h:h + 1], w1slc,
                                 a1sT[pr:pr + F1, h:h + 1])
                nc.tensor.matmul(ps_wt[0:128, 8 + h:9 + h], w1slc,
                                 a1tT[pr:pr + F1, h:h + 1])
            w1t = P.tile([128, 16], F32, tag="w1t")
            nc.vector.tensor_copy(w1t[:], ps_wt[0:128, 0:16])

            # S1T [j(128 x 16 chunks), 16] = x @ w1tilde
            ps_s1t = PS.tile([128, 512], F32, tag="ps")
            for jc in range(NJT):
                nc.tensor.matmul(ps_s1t[0:128, jc * 16:(jc + 1) * 16],
                                 xT[:, jc * 128:(jc + 1) * 128], w1t[:])
            s1T = P.tile([128, NJT * 16], F32, tag="s1T")
            Wvf = P.tile([128, NJT * 16], F32, tag="Wvf")
            Dvf = P.tile([128, NJT * 16], F32, tag="Dvf")
            HC = NJT * 8
            for hh in range(2):
                cs = slice(hh * HC, (hh + 1) * HC)
                nc.vector.tensor_copy(s1T[:, cs], ps_s1t[0:128, cs])
                nc.scalar.activation(Wvf[:, cs], s1T[:, cs], AF.Exp, scale=0.8)
                nc.scalar.activation(Dvf[:, cs], s1T[:, cs], AF.Exp, scale=0.2)

            # s_src rows for our block -> g (bf16) [16, 512]
            ps_s1r = PS.tile([128, 512], F32, tag="ps")
            nc.tensor.matmul(ps_s1r[0:16, 0:RB], w1t[:], xrT[:])
            g1b = P.tile([16, RB], BF16, tag="g1b")
            nc.scalar.activation(g1b[:], ps_s1r[0:16, 0:RB], AF.Exp, scale=-0.8)



            # g broadcasts: PE selector matmul + copy out of PSUM
            gbhs = []
            for h in range(H1):
                ps_g = PS.tile([128, 512], F32, tag="ps")
                nc.tensor.matmul(ps_g[0:128, 0:RB],
                                 sel[:, h * 128:(h + 1) * 128], g1b[:])
                gbh = P.tile([128, RB], BF16, tag=f"gbh{h}", name=f"gbh{h}")
                nc.scalar.activation(gbh[:], ps_g[0:128, 0:RB], AF.Copy)
                gbhs.append(gbh)

            # ============ proj1 -> p1e (+ ones col) =========================
            p1e = P.tile([128, NJT, 8, 66], BF16, tag="p1e")
            nc.vector.memset(p1e[:, :, :, 64:65], 1.0)
            for jt in range(NJT):
                ps_p = PS.tile([128, 512], F32, tag="ps")
                nc.tensor.matmul(ps_p[0:128, 0:512],
                                 xTb[:, jt * 128:(jt + 1) * 128], w1Tb[:])
                dst = p1e[:, jt, :, 0:64]
                src = ps_p[0:128, 0:512].rearrange("p (h q) -> p h q", q=64)
                nc.scalar.activation(dst, src, AF.Copy)

            # ============ layer-1 skip (natural layout, + bias) =============
            # skipsb [128 i, 4 ich, 512 hf] bf16
            skipsb = P.tile([128, 4, H1 * F1], BF16, tag="skipsb")
            for ich in range(NIT):
                ps_sk = PS.tile([128, 512], F32, tag="ps")
                nc.tensor.matmul(ps_sk[0:128, 0:512],
                                 xrTb[:, ich * 128:(ich + 1) * 128], ws1Tb[:],
                                 start=True, stop=False)
                nc.tensor.matmul(ps_sk[0:128, 0:512], ones1b[0:1, 0:128],
                                 b1rb[:], start=False, stop=True)
                nc.scalar.activation(skipsb[:, ich, :], ps_sk[0:128, 0:512],
                                     AF.Copy)

            ps_w2 = PS.tile([128, 512], F32, tag="ps")
            for kt in range(4):
                nc.tensor.matmul(ps_w2[0:128, kt * 2:kt * 2 + 2],
                                 w2n[:, kt * 128:(kt + 1) * 128], a2p[:],
                                 start=True, stop=True)
            w2tb = P.tile([128, 8], BF16, tag="w2tb")
            nc.vector.tensor_copy(w2tb[:], ps_w2[0:128, 0:8])

            # ============ layer-1 head loop (i-part numerator) ==============
            hnat = P.tile([128, 4, H1 * F1], BF16, tag="hnat")
            houtb = P.tile([128, 4, RB], BF16, tag="houtb")
            for h in range(H1):
                ps_h = PSN.tile([128, 4, 65], F32, tag="psn")
                for jp in range(NJT // 2):
                    q = WK.tile([128, 2, RB], BF16, tag="q")
                    for l in range(2):
                        jt = 2 * jp + l
                        col = slice(jt * 16 + 8 + h, jt * 16 + 9 + h)
                        nc.vector.tensor_scalar(q[:, l, :], gbhs[h][:],
                                                Wvf[:, col], Dvf[:, col],
                                                ALU.max, ALU.mult)
                    Ft = WK.tile([128, 2, RB], BF16, tag="F")
                    eng = nc.vector if _mode_a(h, jp) else nc.gpsimd
                    eng.tensor_tensor(Ft[:], q[:],
                                      adjTb[:, 2 * jp:2 * jp + 2, :], ALU.mult)
                    for l in range(2):
                        jt = 2 * jp + l
                        mv = p1e[:, jt, h, 0:65]
                        for ich in range(NIT):
                            nc.tensor.matmul(
                                ps_h[:, ich, 0:65],
                                Ft[:, l, ich * 128:(ich + 1) * 128], mv,
                                start=(jt == 0 and ich == 0),
                                stop=(jt == NJT - 1 and ich == NIT - 1))

                # epilogue: h_nat[:, :, h*64:(h+1)*64] = elu(num/den + skip)
                rec = WK.tile([128, 4], F32, tag="rec")
                nc.vector.reciprocal_approx_fast(
                    rec[:], ps_h[:, :, 64:65].rearrange("p a o -> p (a o)"))
                u1 = WK.tile([128, 4, 64], BF16, tag="u1")
                nc.vector.tensor_tensor(
                    u1[:], ps_h[:, :, 0:64],
                    rec[:].unsqueeze(2).to_broadcast((128, 4, 64)),
                    ALU.mult)
                u = WK.tile([128, 4, 64], BF16, tag="u")
                nc.gpsimd.tensor_tensor(
                    u[:], u1[:],
                    skipsb[:].rearrange("p a (g f) -> p a g f", f=64)[:, :, h, :],
                    ALU.add)
                m0 = WK.tile([128, 4, 64], BF16, tag="m0")
                nc.gpsimd.tensor_scalar(m0[:], u[:], 0.0, 1.0, ALU.min, ALU.mult)
                e = WK.tile([128, 4, 64], BF16, tag="e")
                nc.scalar.activation(e[:], m0[:], AF.Exp)
                nc.vector.scalar_tensor_tensor(
                    hnat[:].rearrange("p a (g f) -> p a g f", f=64)[:, :, h, :],
                    e[:], -1.0, u[:], ALU.add, ALU.max)

                # hnat [128 i, 4 ich, hf] -> houtb (h^T) [128 hf, kt, 512 i],
                # transposed per head-pair as soon as both heads are done
                if h % 2 == 1:
                    kt = h // 2
                    ps_t = PST.tile([128, 512], BF16, tag="pst")
                    for ich in range(NIT):
                        nc.tensor.transpose(
                            ps_t[:, ich * 128:(ich + 1) * 128],
                            hnat[:, ich, kt * 128:(kt + 1) * 128], ident[:])
                    if kt == 3:
                        nc.vector.tensor_copy(houtb[:, kt, :], ps_t[:])
                    else:
                        nc.scalar.activation(houtb[:, kt, :], ps_t[:], AF.Copy)

            # ============ layer-2 local pieces ==============================
            # S2: s_src2 -> psum row 0, s_tgt2 -> psum row 32
            ps_s2 = PS.tile([128, 512], F32, tag="ps")
            for kt in range(4):
                nc.tensor.matmul(ps_s2[0:1, 0:RB], w2tb[:, kt * 2:kt * 2 + 1],
                                 houtb[:, kt, :], start=(kt == 0), stop=(kt == 3))
            for kt in range(4):
                nc.tensor.matmul(ps_s2[32:33, 0:RB], w2tb[:, kt * 2 + 1:kt * 2 + 2],
                                 houtb[:, kt, :], start=(kt == 0), stop=(kt == 3))
            g2row = P.tile([1, RB], BF16, tag="g2row")
            nc.scalar.activation(g2row[:], ps_s2[0:1, 0:RB], AF.Exp, scale=-0.8)
            stg2 = P.tile([1, RB], F32, tag="stg2")
            nc.scalar.activation(stg2[:], ps_s2[32:33, 0:RB], AF.Copy)

            # proj2^T local [64, 512] in bf16 for the gather
            ps_p2 = PS.tile([128, 512], F32, tag="ps")
            for kt in range(4):
                nc.tensor.matmul(ps_p2[0:64, 0:RB], w2Tb[:, kt, :],
                                 houtb[:, kt, :], start=(kt == 0), stop=(kt == 3))
            p2Tb = P.tile([F1, RB], BF16, tag="p2Tb")
            nc.vector.tensor_copy(p2Tb[:], ps_p2[0:64, 0:RB])

            # ============ layer-2 epilogue (natural) ========================
            # skip2 natural [128 i, 64] per ich, with b2 folded in via a
            # rank-1 bias matmul
            b2rowb = P.tile([1, F1], BF16, tag="b2rowb")
            nc.vector.tensor_copy(b2rowb[:], b2row[:])
            sk2sb = P.tile([128, 4, F1], F32, tag="sk2sb")
            for ich in range(NIT):
                pssk = PS.tile([128, 512], F32, tag="ps")
                for kt in range(4):
                    nc.tensor.matmul(pssk[0:128, 0:F1],
                                     houtb[:, kt, ich * 128:(ich + 1) * 128],
                                     ws2Tb[:, kt, :],
                                     start=(kt == 0), stop=False)
                nc.tensor.matmul(pssk[0:128, 0:F1], ones1b[0:1, 0:128],
                                 b2rowb[:], start=False, stop=True)
                nc.vector.tensor_copy(sk2sb[:, ich, :], pssk[0:128, 0:F1])

            ps_g2 = PS.tile([128, 512], F32, tag="ps")
            nc.tensor.matmul(ps_g2[0:128, 0:RB], ones1b[:], g2row[:])
            g2bc = GB.tile([128, RB], BF16, tag="gb")
            nc.vector.tensor_copy(g2bc[:], ps_g2[0:128, 0:RB])

            # ============ AllGather within batch group ======================
            # payload columns: [proj2 (64) | ones (1) | s_tgt2 (1)] so the
            # receiver's moving operand [proj|1] is a contiguous 65-slice
            # and no post-gather memset is needed
            gin = DR.tile([4, F1 + 2, 128], BF16)
            nc.sync.dma_start(
                gin[:, 0:F1, :].rearrange("s f p -> f s p"),
                p2Tb[:].rearrange("f (s p) -> f s p", p=128))
            ones512b = P.tile([1, RB], BF16, tag="ones512b")
            nc.vector.memset(ones512b[:], 1.0)
            nc.sync.dma_start(
                gin[:, F1:F1 + 1, :].rearrange("s o p -> o s p"),
                ones512b[:].rearrange("o (s p) -> o s p", p=128))
            stg2b = P.tile([1, RB], BF16, tag="stg2b")
            nc.vector.tensor_copy(stg2b[:], stg2[:])
            nc.sync.dma_start(
                gin[:, F1 + 1:F1 + 2, :].rearrange("s o p -> o s p"),
                stg2b[:].rearrange("o (s p) -> o s p", p=128))
            gout = DR.tile([4, 4, F1 + 2, 128], BF16)
            nc.gpsimd.collective_compute(
                "AllGather", ALU.bypass, replica_groups=GROUPS,
                ins=[gin.opt()], outs=[gout.opt()])

            # ============ layer-2 attention =================================
            p2e = P.tile([128, NJT, F1 + 2], BF16, tag="p2e")
            nc.sync.dma_start(
                p2e[:, 0:NJT // 2, :],
                gout[0:2].rearrange("c s f p -> p (c s) f"))
            nc.sync.dma_start(
                p2e[:, NJT // 2:NJT, :],
                gout[2:4].rearrange("c s f p -> p (c s) f"))
            st2Tb = P.tile([128, 4, 4], BF16, tag="st2Tb")
            nc.scalar.dma_start(
                st2Tb[:], gout[:, :, F1 + 1, :].rearrange("c s p -> p c s"))
            D2v = P.tile([128, 4, 4], F32, tag="D2v")
            nc.scalar.activation(D2v[:], st2Tb[:], AF.Exp, scale=0.2)
            W2vf = P.tile([128, 4, 4], F32, tag="W2vf")
            nc.scalar.activation(W2vf[:], st2Tb[:], AF.Exp, scale=0.8)

            ps_h2 = PSN.tile([128, 4, 65], F32, tag="psn")
            for jp in range(NJT // 2):
                q2 = WK.tile([128, 2, RB], BF16, tag="q")
                for l in range(2):
                    jt = 2 * jp + l
                    c4, s4 = jt // 4, jt % 4
                    nc.vector.tensor_scalar(q2[:, l, :], g2bc[:],
                                            W2vf[:, c4, s4:s4 + 1],
                                            D2v[:, c4, s4:s4 + 1],
                                            ALU.max, ALU.mult)
                F2 = WK.tile([128, 2, RB], BF16, tag="F")
                eng = nc.gpsimd if jp % 8 < 4 else nc.vector
                eng.tensor_tensor(F2[:], q2[:],
                                  adjTb[:, 2 * jp:2 * jp + 2, :], ALU.mult)
                for l in range(2):
                    jt = 2 * jp + l
                    mv = p2e[:, jt, 0:F1 + 1]
                    for ich in range(NIT):
                        nc.tensor.matmul(
                            ps_h2[:, ich, 0:65],
                            F2[:, l, ich * 128:(ich + 1) * 128], mv,
                            start=(jt == 0 and ich == 0),
                            stop=(jt == NJT - 1 and ich == NIT - 1))

            rec2 = WK.tile([128, 4], F32, tag="rec")
            nc.vector.reciprocal_approx_fast(
                rec2[:], ps_h2[:, :, 64:65].rearrange("p a o -> p (a o)"))
            onat = WK.tile([128, 4, F1], F32, tag="onat")
            t2 = WK.tile([128, 4, F1], F32, tag="t2")
            for hh in range(2):
                ah = slice(2 * hh, 2 * hh + 2)
                nc.vector.tensor_tensor(
                    t2[:, ah, :], ps_h2[:, ah, 0:64],
                    rec2[:, ah].unsqueeze(2).to_broadcast((128, 2, F1)),
                    ALU.mult)
                nc.vector.tensor_tensor(onat[:, ah, :], t2[:, ah, :],
                                        sk2sb[:, ah, :], ALU.add)
                dq = nc.sync if hh == 0 else nc.scalar
                dq.dma_start(
                    d_out[256 * hh:256 * (hh + 1), :]
                    .rearrange("(a p) f -> p a f", p=128),
                    onat[:, ah, :])

    nc.compile()
    return nc


_NC_CACHE = None


def _get_nc():
    global _NC_CACHE
    if _NC_CACHE is None:
        _NC_CACHE = build_nc()
    return _NC_CACHE


def make_in_maps(x, adj, W1, a_src1, a_tgt1, Wskip1, b1, W2, a_src2, a_tgt2,
                 Wskip2, b2):
    import ml_dtypes
    bf16 = ml_dtypes.bfloat16
    x = np.asarray(x, np.float32)
    adj = np.asarray(adj, np.float32)
    W1 = np.asarray(W1, np.float32)
    W2 = np.asarray(W2, np.float32)
    Wskip1 = np.asarray(Wskip1, np.float32)
    Wskip2 = np.asarray(Wskip2, np.float32)
    in_maps = []
    for c in range(NCORES):
        b, r = c // 4, c % 4
        sl = slice(r * RB, (r + 1) * RB)
        xTf = np.ascontiguousarray(x[b].T)
        xrTf = np.ascontiguousarray(x[b][sl].T)
        in_maps.append({
            "xT": xTf, "xTb": xTf.astype(bf16),
            "xrT": xrTf, "xrTb": xrTf.astype(bf16),
            "adjT": np.ascontiguousarray(adj[b][sl].T).astype(bf16),
            "w1": W1,
            "w1Tb": np.ascontiguousarray(W1.T).astype(bf16),
            "ws1Tb": np.ascontiguousarray(Wskip1.T).astype(bf16),
            "asrc1": np.asarray(a_src1, np.float32),
            "atgt1": np.asarray(a_tgt1, np.float32),
            "b1r": np.asarray(b1, np.float32).reshape(1, -1).astype(bf16),
            "w2": W2,
            "w2Tb": np.ascontiguousarray(W2.T).astype(bf16),
            "ws2Tb": np.ascontiguousarray(Wskip2.T).astype(bf16),
            "asrc2": np.asarray(a_src2, np.float32),
            "atgt2": np.asarray(a_tgt2, np.float32),
            "b2": np.asarray(b2, np.float32),
        })
    return in_maps


def kernel(x, adj, W1, a_src1, a_tgt1, Wskip1, b1, W2, a_src2, a_tgt2,
           Wskip2, b2):
    nc = _get_nc()
    in_maps = make_in_maps(x, adj, W1, a_src1, a_tgt1, Wskip1, b1, W2,
                           a_src2, a_tgt2, Wskip2, b2)
    res = run_bass_kernel_spmd(nc, in_maps, core_ids=list(range(NCORES)))
    out = np.empty((BS, N, F1), np.float32)
    for c in range(NCORES):
        b, r = c // 4, c % 4
        out[b, r * RB:(r + 1) * RB, :] = res.results[c]["outN"]
    return out


# revision 50
# speedup vs baseline: 1.3450x; 1.0009x over previous
"""GAT (2-layer graph attention) Trainium2 Bass kernel, 8-core SPMD.

Sharding: data-parallel over batch (2) x row-blocks (4) -> 8 cores.
Core c handles batch b=c//4, output rows R=[512*(c%4), 512*(c%4+1)).

Key algebra: with z = s_src[i]+s_tgt[j], the GAT edge weight
exp(leaky_relu(z, 0.2)) = max(exp(z), exp(0.2 z)).  Softmax rows are
invariant to a per-row scale, so dividing row i by exp(s_src[i]) gives
unnormalized weights F[j,i] = adj[i,j] * D[j] * max(W[j], g[i]) with
  W[j] = exp(0.8 s_tgt[j]),  D[j] = exp(0.2 s_tgt[j]),  g[i] = exp(-0.8 s_src[i])

Layout strategy (v2):
- Host passes adj column-slices pre-transposed and pre-cast to bf16
  (adj is 0/1 so the cast is exact), plus x^T and all weight transposes,
  so no on-chip transposes/casts of inputs are needed.
- The n x n inner work per (head, j-tile) is either one fused
  scalar_tensor_tensor on GPSIMD (max with W, mult by adjT; D folded
  into the stationary operand) or a tensor_scalar + tensor_tensor pair
  on DVE.  g broadcasts ride the idle SP DMA queue.
- The numerator matmul is i-partitioned: stationary = F chunks
  [128j x 128i], moving = proj rows [128j x 65], accumulating into one
  PSUM bank per head ([128, 4, 65]); column 64 gives the softmax
  denominator, which is then a per-partition scalar in the epilogue
  (no reciprocal-broadcast matmuls).

Layer-1 outputs are exchanged within each batch group of 4 cores via a
single AllGather of (proj2^T | s_tgt2) so layer 2 stays row-local.
"""

import os
import sys

for _p in ("/opt/trn_rl_repo", "/root/.axon_site/_ro/trn_rl_repo"):
    if os.path.isdir(_p) and _p not in sys.path:
        sys.path.insert(0, _p)

import numpy as np

import concourse.bass as bass
import concourse.bacc as bacc
import concourse.mybir as mybir
from concourse import tile
from concourse.bass_utils import run_bass_kernel_spmd

F32 = mybir.dt.float32
BF16 = mybir.dt.bfloat16
AF = mybir.ActivationFunctionType
ALU = mybir.AluOpType

BS, N, FIN = 2, 2048, 128
H1, F1 = 8, 64
RB = 512          # row block per core
NJT = N // 128    # 16 j-tiles
NIT = RB // 128   # 4 i-tiles in the row block
NCORES = 8
GROUPS = [[0, 1, 2, 3], [4, 5, 6, 7]]


def _mode_a(h, jp):
    # 'a' pairs: TSP+TT both on DVE; others: TSP on DVE, TT on Pool
    return ((h * 8 + jp) * 7) % 32 < 10


def build_nc():
    nc = bacc.Bacc("TRN2", target_bir_lowering=False, debug=False,
                   num_devices=NCORES)

    # ---- per-core DRAM I/O (host pre-transposes / pre-casts) ----
    d_xT = nc.declare_dram_parameter("xT", [FIN, N], F32, isOutput=False)
    d_xTb = nc.declare_dram_parameter("xTb", [FIN, N], BF16, isOutput=False)
    d_xrT = nc.declare_dram_parameter("xrT", [FIN, RB], F32, isOutput=False)
    d_xrTb = nc.declare_dram_parameter("xrTb", [FIN, RB], BF16, isOutput=False)
    d_adjT = nc.declare_dram_parameter("adjT", [N, RB], BF16, isOutput=False)
    d_w1n = nc.declare_dram_parameter("w1", [H1 * F1, FIN], F32, isOutput=False)
    d_w1Tb = nc.declare_dram_parameter("w1Tb", [FIN, H1 * F1], BF16,
                                       isOutput=False)
    d_ws1Tb = nc.declare_dram_parameter("ws1Tb", [FIN, H1 * F1], BF16,
                                        isOutput=False)
    d_as1 = nc.declare_dram_parameter("asrc1", [H1, F1], F32, isOutput=False)
    d_at1 = nc.declare_dram_parameter("atgt1", [H1, F1], F32, isOutput=False)
    d_b1r = nc.declare_dram_parameter("b1r", [1, H1 * F1], BF16, isOutput=False)
    d_w2 = nc.declare_dram_parameter("w2", [F1, H1 * F1], F32, isOutput=False)
    d_w2Tb = nc.declare_dram_parameter("w2Tb", [H1 * F1, F1], BF16,
                                       isOutput=False)
    d_ws2Tb = nc.declare_dram_parameter("ws2Tb", [H1 * F1, F1], BF16,
                                        isOutput=False)
    d_as2 = nc.declare_dram_parameter("asrc2", [1, F1], F32, isOutput=False)
    d_at2 = nc.declare_dram_parameter("atgt2", [1, F1], F32, isOutput=False)
    d_b2 = nc.declare_dram_parameter("b2", [F1], F32, isOutput=False)
    # output: natural row-block [512, 64]
    d_out = nc.declare_dram_parameter("outN", [RB, F1], F32, isOutput=True)

    with tile.TileContext(nc) as tc:
        with (
            tc.tile_pool(name="persist", bufs=1) as P,
            tc.tile_pool(name="work", bufs=8) as WK,
            tc.tile_pool(name="gbp", bufs=3) as GB,
            tc.tile_pool(name="ps", bufs=3, space="PSUM") as PS,
            tc.tile_pool(name="psnum", bufs=4, space="PSUM") as PSN,
            tc.tile_pool(name="pst", bufs=1, space="PSUM") as PST,
            tc.tile_pool(name="dram", bufs=1, space="DRAM") as DR,
        ):
            # ============ loads (all natural-layout now) ====================
            # SP queue: xT first (scores path), then xTb, then adjT half
            xT = P.tile([128, N], F32, tag="xT")
            xTb = P.tile([128, N], BF16, tag="xTb")
            nc.sync.dma_start(xT[:, 0:N // 2], d_xT[:, 0:N // 2])
            nc.sync.dma_start(xTb[:, 0:N // 2], d_xTb[:, 0:N // 2])
            nc.sync.dma_start(xT[:, N // 2:N], d_xT[:, N // 2:N])
            nc.sync.dma_start(xTb[:, N // 2:N], d_xTb[:, N // 2:N])
            # Act queue: adjT other half + xr + small weights
            adjTb = P.tile([128, NJT, RB], BF16, tag="adjTb")
            nc.sync.dma_start(
                adjTb[:, 0:NJT // 2, :],
                d_adjT[0:N // 2, :].rearrange("(t p) i -> p t i", p=128))
            nc.sync.dma_start(
                adjTb[:, NJT // 2:NJT, :],
                d_adjT[N // 2:N, :].rearrange("(t p) i -> p t i", p=128))
            xrT = P.tile([128, RB], F32, tag="xrT")
            nc.scalar.dma_start(xrT[:], d_xrT[:, :])
            xrTb = P.tile([128, RB], BF16, tag="xrTb")
            nc.scalar.dma_start(xrTb[:], d_xrTb[:, :])
            # selector tiles sel_h [16, 128] (row h ones) for g broadcasts,
            # emitted before the Pool DMA chain so they clear the queue early
            ones16 = P.tile([16, 128], BF16, tag="ones16")
            nc.vector.memset(ones16[:], 1.0)
            sel = P.tile([16, H1 * 128], BF16, tag="sel")
            for h in range(H1):
                nc.gpsimd.affine_select(sel[:, h * 128:(h + 1) * 128],
                                        ones16[:], [[0, 128]], ALU.is_equal,
                                        0.0, base=-h, channel_multiplier=1)
            # Pool queue: proj1 weights first, then a-vectors
            w1Tb = P.tile([128, H1 * F1], BF16, tag="w1Tb")
            nc.gpsimd.dma_start(w1Tb[:], d_w1Tb[:, :])
            w1n = P.tile([128, 4, FIN], F32, tag="w1n")
            nc.gpsimd.dma_start(w1n[:], d_w1n.rearrange("(k p) c -> p k c", p=128))
            a1sT = P.tile([128, H1], F32, tag="a1sT")
            nc.gpsimd.dma_start(a1sT[0:F1, :], d_as1.rearrange("h f -> f h"))
            nc.gpsimd.dma_start(a1sT[F1:2 * F1, :], d_as1.rearrange("h f -> f h"))
            a1tT = P.tile([128, H1], F32, tag="a1tT")
            nc.gpsimd.dma_start(a1tT[0:F1, :], d_at1.rearrange("h f -> f h"))
            nc.gpsimd.dma_start(a1tT[F1:2 * F1, :], d_at1.rearrange("h f -> f h"))
            ws1Tb = P.tile([128, H1 * F1], BF16, tag="ws1Tb")
            nc.gpsimd.dma_start(ws1Tb[:], d_ws1Tb[:, :])
            b1rb = P.tile([1, H1 * F1], BF16, tag="b1rb")
            nc.sync.dma_start(b1rb[:], d_b1r[:, :])
            # L2 weights ride the SP queue later (needed only at L2 time)
            w2n = P.tile([F1, H1 * F1], F32, tag="w2n")
            nc.sync.dma_start(w2n[:], d_w2[:, :])
            w2Tb = P.tile([128, 4, F1], BF16, tag="w2Tb")
            nc.sync.dma_start(w2Tb[:], d_w2Tb.rearrange("(k p) f -> p k f", p=128))
            ws2Tb = P.tile([128, 4, F1], BF16, tag="ws2Tb")
            nc.sync.dma_start(ws2Tb[:], d_ws2Tb.rearrange("(k p) f -> p k f", p=128))
            a2p = P.tile([F1, 2], F32, tag="a2p")
            nc.sync.dma_start(a2p[:, 0:1], d_as2.rearrange("o f -> f o"))
            nc.sync.dma_start(a2p[:, 1:2], d_at2.rearrange("o f -> f o"))
            b2row = P.tile([1, F1], F32, tag="b2row")
            nc.sync.dma_start(b2row[:], d_b2.ap().rearrange("(o f) -> o f", o=1))

            ones1b = P.tile([1, 128], BF16, tag="ones1b")
            nc.vector.memset(ones1b[:], 1.0)
            # identity (bf16) for PE transposes of h
            onesq = P.tile([128, 128], BF16, tag="onesq")
            nc.vector.memset(onesq[:], 1.0)
            ident = P.tile([128, 128], BF16, tag="ident")
            nc.gpsimd.affine_select(ident[:], onesq[:], [[-1, 128]],
                                    ALU.is_equal, 0.0, base=0,
                                    channel_multiplier=1)

            # ============ scores (exact fp32) ===============================
            # w1tilde [c=128, 16]: col h = W1_h^T a_src1[h], col 8+h tgt
            ps_wt = PS.tile([128, 512], F32, tag="ps")
            for h in range(H1):
                kt, pr = (h * F1) // 128, (h * F1) % 128
                w1slc = w1n[pr:pr + F1, kt, :]
                nc.tensor.matmul(ps_wt[0:128, h:h + 1], w1slc,
                                 a1sT[pr:pr + F1, h:h + 1])
                nc.tensor.matmul(ps_wt[0:128, 8 + h:9 + h], w1slc,
                                 a1tT[pr:pr + F1, h:h + 1])
            w1t = P.tile([128, 16], F32, tag="w1t")
            nc.vector.tensor_copy(w1t[:], ps_wt[0:128, 0:16])

            # S1T [j(128 x 16 chunks), 16] = x @ w1tilde
            ps_s1t = PS.tile([128, 512], F32, tag="ps")
            for jc in range(NJT):
                nc.tensor.matmul(ps_s1t[0:128, jc * 16:(jc + 1) * 16],
                                 xT[:, jc * 128:(jc + 1) * 128], w1t[:])
            s1T = P.tile([128, NJT * 16], F32, tag="s1T")
            Wvf = P.tile([128, NJT * 16], F32, tag="Wvf")
            Dvf = P.tile([128, NJT * 16], F32, tag="Dvf")
            HC = NJT * 8
            for hh in range(2):
                cs = slice(hh * HC, (hh + 1) * HC)
                nc.vector.tensor_copy(s1T[:, cs], ps_s1t[0:128, cs])
                nc.scalar.activation(Wvf[:, cs], s1T[:, cs], AF.Exp, scale=0.8)
                nc.scalar.activation(Dvf[:, cs], s1T[:, cs], AF.Exp, scale=0.2)

            # s_src rows for our block -> g (bf16) [16, 512]
            ps_s1r = PS.tile([128, 512], F32, tag="ps")
            nc.tensor.matmul(ps_s1r[0:16, 0:RB], w1t[:], xrT[:])
            g1b = P.tile([16, RB], BF16, tag="g1b")
            nc.scalar.activation(g1b[:], ps_s1r[0:16, 0:RB], AF.Exp, scale=-0.8)



            # g broadcasts: PE selector matmul + copy out of PSUM
            gbhs = []
            for h in range(H1):
                ps_g = PS.tile([128, 512], F32, tag="ps")
                nc.tensor.matmul(ps_g[0:128, 0:RB],
                                 sel[:, h * 128:(h + 1) * 128], g1b[:])
                gbh = P.tile([128, RB], BF16, tag=f"gbh{h}", name=f"gbh{h}")
                nc.scalar.activation(gbh[:], ps_g[0:128, 0:RB], AF.Copy)
                gbhs.append(gbh)

            # ============ proj1 -> p1e (+ ones col) =========================
            p1e = P.tile([128, NJT, 8, 66], BF16, tag="p1e")
            nc.vector.memset(p1e[:, :, :, 64:65], 1.0)
            for jt in range(NJT):
                ps_p = PS.tile([128, 512], F32, tag="ps")
                nc.tensor.matmul(ps_p[0:128, 0:512],
                                 xTb[:, jt * 128:(jt + 1) * 128], w1Tb[:])
                dst = p1e[:, jt, :, 0:64]
                src = ps_p[0:128, 0:512].rearrange("p (h q) -> p h q", q=64)
                nc.scalar.activation(dst, src, AF.Copy)

            # ============ layer-1 skip (natural layout, + bias) =============
            # skipsb [128 i, 4 ich, 512 hf] bf16
            skipsb = P.tile([128, 4, H1 * F1], BF16, tag="skipsb")
            for ich in range(NIT):
                ps_sk = PS.tile([128, 512], F32, tag="ps")
                nc.tensor.matmul(ps_sk[0:128, 0:512],
                                 xrTb[:, ich * 128:(ich + 1) * 128], ws1Tb[:],
                                 start=True, stop=False)
                nc.tensor.matmul(ps_sk[0:128, 0:512], ones1b[0:1, 0:128],
                                 b1rb[:], start=False, stop=True)
                nc.scalar.activation(skipsb[:, ich, :], ps_sk[0:128, 0:512],
                                     AF.Copy)

            ps_w2 = PS.tile([128, 512], F32, tag="ps")
            for kt in range(4):
                nc.tensor.matmul(ps_w2[0:128, kt * 2:kt * 2 + 2],
                                 w2n[:, kt * 128:(kt + 1) * 128], a2p[:],
                                 start=True, stop=True)
            w2tb = P.tile([128, 8], BF16, tag="w2tb")
            nc.vector.tensor_copy(w2tb[:], ps_w2[0:128, 0:8])

            # ============ layer-1 head loop (i-part numerator) ==============
            hnat = P.tile([128, 4, H1 * F1], BF16, tag="hnat")
            houtb = P.tile([128, 4, RB], BF16, tag="houtb")
            for h in range(H1):
                ps_h = PSN.tile([128, 4, 65], F32, tag="psn")
                for jp in range(NJT // 2):
                    q = WK.tile([128, 2, RB], BF16, tag="q")
                    for l in range(2):
                        jt = 2 * jp + l
                        col = slice(jt * 16 + 8 + h, jt * 16 + 9 + h)
                        nc.vector.tensor_scalar(q[:, l, :], gbhs[h][:],
                                                Wvf[:, col], Dvf[:, col],
                                                ALU.max, ALU.mult)
                    Ft = WK.tile([128, 2, RB], BF16, tag="F")
                    eng = nc.vector if _mode_a(h, jp) else nc.gpsimd
                    eng.tensor_tensor(Ft[:], q[:],
                                      adjTb[:, 2 * jp:2 * jp + 2, :], ALU.mult)
                    for l in range(2):
                        jt = 2 * jp + l
                        mv = p1e[:, jt, h, 0:65]
                        for ich in range(NIT):
                            nc.tensor.matmul(
                                ps_h[:, ich, 0:65],
                                Ft[:, l, ich * 128:(ich + 1) * 128], mv,
                                start=(jt == 0 and ich == 0),
                                stop=(jt == NJT - 1 and ich == NIT - 1))

                # epilogue: h_nat[:, :, h*64:(h+1)*64] = elu(num/den + skip)
                rec = WK.tile([128, 4], F32, tag="rec")
                nc.vector.reciprocal_approx_fast(
                    rec[:], ps_h[:, :, 64:65].rearrange("p a o -> p (a o)"))
                u1 = WK.tile([128, 4, 64], BF16, tag="u1")
                nc.vector.tensor_tensor(
                    u1[:], ps_h[:, :, 0:64],
                    rec[:].unsqueeze(2).to_broadcast((128, 4, 64)),
                    ALU.mult)
                u = WK.tile([128, 4, 64], BF16, tag="u")
                nc.gpsimd.tensor_tensor(
                    u[:], u1[:],
                    skipsb[:].rearrange("p a (g f) -> p a g f", f=64)[:, :, h, :],
                    ALU.add)
                m0 = WK.tile([128, 4, 64], BF16, tag="m0")
                nc.gpsimd.tensor_scalar(m0[:], u[:], 0.0, 1.0, ALU.min, ALU.mult)
                e = WK.tile([128, 4, 64], BF16, tag="e")
                nc.scalar.activation(e[:], m0[:], AF.Exp)
                nc.vector.scalar_tensor_tensor(
                    hnat[:].rearrange("p a (g f) -> p a g f", f=64)[:, :, h, :],
                    e[:], -1.0, u[:], ALU.add, ALU.max)

                # hnat [128 i, 4 ich, hf] -> houtb (h^T) [128 hf, kt, 512 i],
                # transposed per head-pair as soon as both heads are done
                if h % 2 == 1:
                    kt = h // 2
                    ps_t = PST.tile([128, 512], BF16, tag="pst")
                    for ich in range(NIT):
                        nc.tensor.transpose(
                            ps_t[:, ich * 128:(ich + 1) * 128],
                            hnat[:, ich, kt * 128:(kt + 1) * 128], ident[:])
                    if kt == 3:
                        nc.vector.tensor_copy(houtb[:, kt, :], ps_t[:])
                    else:
                        nc.scalar.activation(houtb[:, kt, :], ps_t[:], AF.Copy)

            # ============ layer-2 local pieces ==============================
            # S2: s_src2 -> psum row 0, s_tgt2 -> psum row 32
            ps_s2 = PS.tile([128, 512], F32, tag="ps")
            for kt in range(4):
                nc.tensor.matmul(ps_s2[0:1, 0:RB], w2tb[:, kt * 2:kt * 2 + 1],
                                 houtb[:, kt, :], start=(kt == 0), stop=(kt == 3))
            for kt in range(4):
                nc.tensor.matmul(ps_s2[32:33, 0:RB], w2tb[:, kt * 2 + 1:kt * 2 + 2],
                                 houtb[:, kt, :], start=(kt == 0), stop=(kt == 3))
            g2row = P.tile([1, RB], BF16, tag="g2row")
            nc.scalar.activation(g2row[:], ps_s2[0:1, 0:RB], AF.Exp, scale=-0.8)
            stg2 = P.tile([1, RB], F32, tag="stg2")
            nc.scalar.activation(stg2[:], ps_s2[32:33, 0:RB], AF.Copy)

            # proj2^T local [64, 512] in bf16 for the gather
            ps_p2 = PS.tile([128, 512], F32, tag="ps")
            for kt in range(4):
                nc.tensor.matmul(ps_p2[0:64, 0:RB], w2Tb[:, kt, :],
                                 houtb[:, kt, :], start=(kt == 0), stop=(kt == 3))
            p2Tb = P.tile([F1, RB], BF16, tag="p2Tb")
            nc.vector.tensor_copy(p2Tb[:], ps_p2[0:64, 0:RB])

            # ============ layer-2 epilogue (natural) ========================
            # skip2 natural [128 i, 64] per ich, with b2 folded in via a
            # rank-1 bias matmul
            b2rowb = P.tile([1, F1], BF16, tag="b2rowb")
            nc.vector.tensor_copy(b2rowb[:], b2row[:])
            sk2sb = P.tile([128, 4, F1], F32, tag="sk2sb")
            for ich in range(NIT):
                pssk = PS.tile([128, 512], F32, tag="ps")
                for kt in range(4):
                    nc.tensor.matmul(pssk[0:128, 0:F1],
                                     houtb[:, kt, ich * 128:(ich + 1) * 128],
                                     ws2Tb[:, kt, :],
                                     start=(kt == 0), stop=False)
                nc.tensor.matmul(pssk[0:128, 0:F1], ones1b[0:1, 0:128],
                                 b2rowb[:], start=False, stop=True)
                nc.vector.tensor_copy(sk2sb[:, ich, :], pssk[0:128, 0:F1])

            ps_g2 = PS.tile([128, 512], F32, tag="ps")
            nc.tensor.matmul(ps_g2[0:128, 0:RB], ones1b[:], g2row[:])
            g2bc = GB.tile([128, RB], BF16, tag="gb")
            nc.vector.tensor_copy(g2bc[:], ps_g2[0:128, 0:RB])

            # ============ AllGather within batch group ======================
            # payload columns: [proj2 (64) | ones (1) | s_tgt2 (1)] so the
            # receiver's moving operand [proj|1] is a contiguous 65-slice
            # and no post-gather memset is needed
            gin = DR.tile([4, F1 + 2, 128], BF16)
            nc.sync.dma_start(
                gin[:, 0:F1, :].rearrange("s f p -> f s p"),
                p2Tb[:].rearrange("f (s p) -> f s p", p=128))
            ones512b = P.tile([1, RB], BF16, tag="ones512b")
            nc.vector.memset(ones512b[:], 1.0)
            nc.sync.dma_start(
                gin[:, F1:F1 + 1, :].rearrange("s o p -> o s p"),
                ones512b[:].rearrange("o (s p) -> o s p", p=128))
            stg2b = P.tile([1, RB], BF16, tag="stg2b")
            nc.vector.tensor_copy(stg2b[:], stg2[:])
            nc.sync.dma_start(
                gin[:, F1 + 1:F1 + 2, :].rearrange("s o p -> o s p"),
                stg2b[:].rearrange("o (s p) -> o s p", p=128))
            gout = DR.tile([4, 4, F1 + 2, 128], BF16)
            nc.gpsimd.collective_compute(
                "AllGather", ALU.bypass, replica_groups=GROUPS,
                ins=[gin.opt()], outs=[gout.opt()])

            # ============ layer-2 attention =================================
            p2e = P.tile([128, NJT, F1 + 2], BF16, tag="p2e")
            nc.sync.dma_start(
                p2e[:, 0:NJT // 2, :],
                gout[0:2].rearrange("c s f p -> p (c s) f"))
            nc.sync.dma_start(
                p2e[:, NJT // 2:NJT, :],
                gout[2:4].rearrange("c s f p -> p (c s) f"))
            st2Tb = P.tile([128, 4, 4], BF16, tag="st2Tb")
            nc.scalar.dma_start(
                st2Tb[:], gout[:, :, F1 + 1, :].rearrange("c s p -> p c s"))
            D2v = P.tile([128, 4, 4], F32, tag="D2v")
            nc.scalar.activation(D2v[:], st2Tb[:], AF.Exp, scale=0.2)
            W2vf = P.tile([128, 4, 4], F32, tag="W2vf")
            nc.scalar.activation(W2vf[:], st2Tb[:], AF.Exp, scale=0.8)

            ps_h2 = PSN.tile([128, 4, 65], F32, tag="psn")
            for jp in range(NJT // 2):
                q2 = WK.tile([128, 2, RB], BF16, tag="q")
                for l in range(2):
                    jt = 2 * jp + l
                    c4, s4 = jt // 4, jt % 4
                    nc.vector.tensor_scalar(q2[:, l, :], g2bc[:],
                                            W2vf[:, c4, s4:s4 + 1],
                                            D2v[:, c4, s4:s4 + 1],
                                            ALU.max, ALU.mult)
                F2 = WK.tile([128, 2, RB], BF16, tag="F")
                eng = nc.gpsimd if jp % 8 < 4 else nc.vector
                eng.tensor_tensor(F2[:], q2[:],
                                  adjTb[:, 2 * jp:2 * jp + 2, :], ALU.mult)
                for l in range(2):
                    jt = 2 * jp + l
                    mv = p2e[:, jt, 0:F1 + 1]
                    for ich in range(NIT):
                        nc.tensor.matmul(
                            ps_h2[:, ich, 0:65],
                            F2[:, l, ich * 128:(ich + 1) * 128], mv,
                            start=(jt == 0 and ich == 0),
                            stop=(jt == NJT - 1 and ich == NIT - 1))

            rec2 = WK.tile([128, 4], F32, tag="rec")
            nc.vector.reciprocal_approx_fast(
                rec2[:], ps_h2[:, :, 64:65].rearrange("p a o -> p (a o)"))
            onat = WK.tile([128, 4, F1], F32, tag="onat")
            t2 = WK.tile([128, 4, F1], F32, tag="t2")
            for hh in range(2):
                ah = slice(2 * hh, 2 * hh + 2)
                nc.vector.tensor_tensor(
                    t2[:, ah, :], ps_h2[:, ah, 0:64],
                    rec2[:, ah].unsqueeze(2).to_broadcast((128, 2, F1)),
                    ALU.mult)
                nc.vector.tensor_tensor(onat[:, ah, :], t2[:, ah, :],
                                        sk2sb[:, ah, :], ALU.add)
                dq = nc.sync if hh == 0 else nc.scalar
                dq.dma_start(
                    d_out[256 * hh:256 * (hh + 1), :]
                    .rearrange("(a p) f -> p a f", p=128),
                    onat[:, ah, :])

    nc.compile()
    return nc


_NC_CACHE = None


def _get_nc():
    global _NC_CACHE
    if _NC_CACHE is None:
        _NC_CACHE = build_nc()
    return _NC_CACHE


def make_in_maps(x, adj, W1, a_src1, a_tgt1, Wskip1, b1, W2, a_src2, a_tgt2,
                 Wskip2, b2):
    import ml_dtypes
    bf16 = ml_dtypes.bfloat16
    x = np.asarray(x, np.float32)
    adj = np.asarray(adj, np.float32)
    W1 = np.asarray(W1, np.float32)
    W2 = np.asarray(W2, np.float32)
    Wskip1 = np.asarray(Wskip1, np.float32)
    Wskip2 = np.asarray(Wskip2, np.float32)
    in_maps = []
    for c in range(NCORES):
        b, r = c // 4, c % 4
        sl = slice(r * RB, (r + 1) * RB)
        xTf = np.ascontiguousarray(x[b].T)
        xrTf = np.ascontiguousarray(x[b][sl].T)
        in_maps.append({
            "xT": xTf, "xTb": xTf.astype(bf16),
            "xrT": xrTf, "xrTb": xrTf.astype(bf16),
            "adjT": np.ascontiguousarray(adj[b][sl].T).astype(bf16),
            "w1": W1,
            "w1Tb": np.ascontiguousarray(W1.T).astype(bf16),
            "ws1Tb": np.ascontiguousarray(Wskip1.T).astype(bf16),
            "asrc1": np.asarray(a_src1, np.float32),
            "atgt1": np.asarray(a_tgt1, np.float32),
            "b1r": np.asarray(b1, np.float32).reshape(1, -1).astype(bf16),
            "w2": W2,
            "w2Tb": np.ascontiguousarray(W2.T).astype(bf16),
            "ws2Tb": np.ascontiguousarray(Wskip2.T).astype(bf16),
            "asrc2": np.asarray(a_src2, np.float32),
            "atgt2": np.asarray(a_tgt2, np.float32),
            "b2": np.asarray(b2, np.float32),
        })
    return in_maps


def kernel(x, adj, W1, a_src1, a_tgt1, Wskip1, b1, W2, a_src2, a_tgt2,
           Wskip2, b2):
    nc = _get_nc()
    in_maps = make_in_maps(x, adj, W1, a_src1, a_tgt1, Wskip1, b1, W2,
                           a_src2, a_tgt2, Wskip2, b2)
    res = run_bass_kernel_spmd(nc, in_maps, core_ids=list(range(NCORES)))
    out = np.empty((BS, N, F1), np.float32)
    for c in range(NCORES):
        b, r = c // 4, c % 4
        out[b, r * RB:(r + 1) * RB, :] = res.results[c]["outN"]
    return out
